# revision 1
# baseline (speedup 1.0000x reference)
"""Multi-head attention (axis-swapped variant) on 8 Trainium2 NeuronCores.

Reference semantics (EMB=1024, heads act on the d_head axis after the buggy
transpose): 64 effective heads of size 16, causal softmax scaled by
1/sqrt(16), projections Wq/Wk/Wv, output projection Wo + bo.

Sharding: core c = 4*b + g handles batch b and head-group g (16 heads =
256 contiguous projection columns). Each core returns a partial output
[1024, 1024]; the host sums the 4 group partials per batch and adds bo.
"""

import numpy as np

import concourse.bass as bass
import concourse.mybir as mybir
import concourse.tile as tile
from concourse.bass_utils import run_bass_kernel_spmd

F32 = mybir.dt.float32
F32R = mybir.dt.float32r
BF16 = mybir.dt.bfloat16

EMB = 1024
SEQ = 1024
BATCH = 2
NG = 4            # head groups (cores per batch)
HPG = 16          # heads per group/core
DH = 16           # per-head feature size
GCOLS = HPG * DH  # 256 projection columns per core


def split_excess_waits(nc, cap=1):
    """This container's walrus rejects instructions carrying more than a few
    semaphore waits (and bass's own model says one). Relocate excess waits
    onto preceding same-engine EventSemaphore instructions."""

    def fix_block(bb, dummy):
        insts = bb.instructions
        i = 0
        while i < len(insts):
            inst = insts[i]
            si = inst.sync_info
            waits = list(si.on_wait) if si is not None and si.on_wait else []
            if len(waits) > cap:
                eng = nc.engines[inst.engine]
                excess, keep = waits[:-cap], waits[-cap:]
                si.on_wait = keep
                pos = i
                for j in range(0, len(excess), cap):
                    chunk = excess[j : j + cap]
                    ev = eng.wait_ge(dummy, 1)
                    cur_list = nc.cur_bb.bb.instructions
                    assert cur_list[-1] is ev.ins
                    cur_list.pop()
                    ev.ins.sync_info.on_wait = chunk
                    insts.insert(pos, ev.ins)
                    pos += 1
                    i += 1
            i += 1

    with nc.semaphore("waitfix_dummy") as dummy:
        for f in nc.m.functions:
            for bb in f.blocks:
                fix_block(bb, dummy)


def _r(ap):
    return ap.bitcast(F32R)


def build_nc():
    nc = bass.Bass()
    xT_d = nc.declare_dram_parameter("xT", [EMB, SEQ], F32R, isOutput=False)
    wq_d = nc.declare_dram_parameter("wq", [EMB, GCOLS], F32R, isOutput=False)
    wk_d = nc.declare_dram_parameter("wk", [EMB, GCOLS], F32R, isOutput=False)
    wv_d = nc.declare_dram_parameter("wv", [EMB, GCOLS], F32R, isOutput=False)
    wo_d = nc.declare_dram_parameter("wo", [512, EMB], F32R, isOutput=False)
    y_d = nc.declare_dram_parameter("y", [SEQ, EMB], F32, isOutput=True)


    with tile.TileContext(nc) as tc:
        with (
            tc.tile_pool(name="big", bufs=1) as big,
            tc.tile_pool(name="work", bufs=3) as work,
            tc.tile_pool(name="att", bufs=6) as att,
            tc.tile_pool(name="dram", bufs=1, space="DRAM") as dram,
        ):
            drc = dram.tile([HPG, SEQ], F32)   # gathered denominators
            drr = dram.tile([HPG, SEQ], F32)   # their reciprocals
            # ---- stage A: load everything (weights first, xT chunked
            # so the first projection matmuls start early) ----
            XT = big.tile([128, 8, SEQ], F32R)      # [d_in_block, kb, m]
            xT_r = xT_d[:].rearrange("(kb p) m -> p kb m", p=128)
            WQ = big.tile([128, 8, GCOLS], F32R)
            nc.sync.dma_start(WQ[:], wq_d[:].rearrange("(kb p) n -> p kb n", p=128))
            for kb in range(8):
                nc.sync.dma_start(XT[:, kb, :], xT_r[:, kb, :])
            WK = big.tile([128, 8, GCOLS], F32R)
            nc.sync.dma_start(WK[:], wk_d[:].rearrange("(kb p) n -> p kb n", p=128))
            WV = big.tile([128, 8, GCOLS], F32R)
            nc.sync.dma_start(WV[:], wv_d[:].rearrange("(kb p) n -> p kb n", p=128))
            WO = big.tile([128, 4, EMB], F32R)
            nc.sync.dma_start(WO[:], wo_d[:].rearrange("(t p) n -> p t n", p=128))

            # v-augmented weights: per (jb, head) a [128, 32] block =
            # [v(16 cols) | 1 | zeros(15)]
            VA = big.tile([128, 8, HPG, 32], BF16)
            nc.gpsimd.memset(VA[:], 0.0)
            nc.gpsimd.memset(VA[:, :, :, 16:17], 1.0)

            # ---- stage B: projections (fp32r) ----
            # qT/kT: out[col, m] = sum_d W[d, col] * xT[d, m]
            QT = big.tile([128, 2, SEQ], BF16)
            KT = big.tile([128, 2, SEQ], BF16)
            ps_proj_cm = tc.tile_pool(name="ps_proj", bufs=2, space="PSUM")
            ps_proj = ps_proj_cm.__enter__()
            for W, T in ((WQ, QT), (WK, KT)):
                for t in range(2):
                    for ic in range(2):
                        pq = ps_proj.tile([128, 512], F32, tag="pproj")
                        for kb in range(8):
                            nc.tensor.matmul(
                                pq[:],
                                W[:, kb, 128 * t : 128 * t + 128],
                                XT[:, kb, 512 * ic : 512 * ic + 512],
                                start=(kb == 0),
                                stop=(kb == 7),
                            )
                        nc.vector.tensor_copy(
                            T[:, t, 512 * ic : 512 * ic + 512], pq[:]
                        )
            # v natural: out[m, col] = sum_d xT[d, m] * W[d, col]; write
            # straight into VA's per-head v columns (bf16 cast on copy).
            for mt in range(8):
                pv = ps_proj.tile([128, GCOLS], F32, tag="pproj")
                for kb in range(8):
                    nc.tensor.matmul(
                        pv[:],
                        XT[:, kb, 128 * mt : 128 * mt + 128],
                        WV[:, kb, :],
                        start=(kb == 0),
                        stop=(kb == 7),
                    )
                nc.vector.tensor_copy(
                    VA[:, mt, :, 0:16],
                    pv[:].rearrange("p (h e) -> p h e", e=DH),
                )

            ps_proj_cm.__exit__(None, None, None)

            # shifted copies: odd heads moved down 16 partitions so every
            # head's 16 rows start at a 32-aligned partition
            QTs = big.tile([128, 2, SEQ], BF16)
            KTs = big.tile([128, 2, SEQ], BF16)
            for src, dst in ((QT, QTs), (KT, KTs)):
                for j in range(4):
                    nc.sync.dma_start(
                        dst[32 * j : 32 * j + 16, :, :],
                        src[32 * j + 16 : 32 * j + 32, :, :],
                    )

            def head_slice(T, Ts, h, lo, size):
                t, hl = divmod(h, 8)
                src = T if hl % 2 == 0 else Ts
                base = 16 * (hl - hl % 2)
                return src[base : base + 16, t, lo : lo + size]

            # ---- stage C: attention ----
            # i-chunk-outer so ctx PSUM is 1 bank; scores tiles 2 banks x3
            # bufs so PE/ACT/Pool pipeline across (pair, jb)
            SCr = []
            SC = []
            for t2 in range(4):
                SCr.append(big.tile([128, SEQ], F32, tag=f"scr{t2}", name=f"scr{t2}"))
            with (
                tc.tile_pool(name="ps_sw", bufs=3, space="PSUM") as ps_sw,
                tc.tile_pool(name="ps_cp", bufs=2, space="PSUM") as ps_cp,
            ):
                for t2 in range(4):
                    # quad heads {0,2,4,6}+off: one source tile, bases
                    # {0,32,64,96} -> 4 concurrent row-groups on the PE
                    toff = (t2 // 2) * 8
                    quad = [toff + 2 * q + (t2 % 2) for q in range(4)]
                    for ic in range(2):
                        c0 = 512 * ic
                        CP = ps_cp.tile([128, 512], F32, tag="cp")
                        bank_first = [True] * 4
                        for jb in range(2 * (ic + 1) * 2):
                            i0 = 128 * jb
                            lo = max(c0, i0)
                            hi = c0 + 512
                            if lo >= hi:
                                continue
                            SWs, ATs = [], []
                            for pr in range(2):
                                SW = ps_sw.tile([128, 2, 512], F32, tag="sw")
                                AT = att.tile([128, 2, 512], BF16, tag="at")
                                SWs.append(SW)
                                ATs.append(AT)
                                for ph in range(2):
                                    h = quad[2 * pr + ph]
                                    hl = h % 8
                                    tp = (
                                        (96, 0)
                                        if 16 * (hl - hl % 2) == 96
                                        else None
                                    )
                                    nc.tensor.matmul(
                                        SW[:, ph, lo - c0 : hi - c0],
                                        head_slice(KT, KTs, h, i0, 128),
                                        head_slice(QT, QTs, h, lo, hi - lo),
                                        start=True,
                                        stop=True,
                                        tile_position=tp,
                                    )
                            for pr in range(2):
                                nc.scalar.activation(
                                    ATs[pr][:, :, lo - c0 : 512],
                                    SWs[pr][:, :, lo - c0 : 512],
                                    mybir.ActivationFunctionType.Exp,
                                    scale=0.25,
                                )
                            if lo == i0:
                                for pr in range(2):
                                    # diagonal block: keep j <= i
                                    nc.gpsimd.affine_select(
                                        out=ATs[pr][:, :, lo - c0 : lo - c0 + 128],
                                        in_=ATs[pr][:, :, lo - c0 : lo - c0 + 128],
                                        compare_op=mybir.AluOpType.is_ge,
                                        fill=0.0,
                                        base=0,
                                        pattern=[[0, 2], [1, 128]],
                                        channel_multiplier=-1,
                                    )
                            for pr in range(2):
                                for ph in range(2):
                                    h = quad[2 * pr + ph]
                                    cg = 2 * pr + ph
                                    nc.tensor.matmul(
                                        CP[32 * cg : 32 * cg + 32, lo - c0 : 512],
                                        VA[:, jb, h, :],
                                        ATs[pr][:, ph, lo - c0 : 512],
                                        start=bank_first[cg],
                                        stop=False,
                                        tile_position=(0, 32 * cg),
                                        skip_group_check=True,
                                    )
                                    bank_first[cg] = False
                        # evacuate this i-chunk of ctxT_aug
                        nc.vector.tensor_copy(
                            SCr[t2][:, c0 : c0 + 512], CP[:]
                        )
                    # gather the 4 denominator rows {16,48,80,112} -> DRAM
                    s = SCr[t2][:]
                    nc.sync.dma_start(
                        drc[4 * t2 : 4 * t2 + 4, :],
                        bass.AP(
                            tensor=s.tensor,
                            offset=s[16:17].offset,
                            ap=[[32 * s.ap[0][0], 4]] + s[16:17].ap[1:],
                        ),
                    )
                    # per-quad reciprocal + broadcast + divide, overlapping
                    # the next quad's attention
                    DSQ = work.tile([32, 128], F32, tag="dsq")
                    nc.sync.dma_start(
                        DSQ[:],
                        drc[4 * t2 : 4 * t2 + 4, :].rearrange(
                            "h (pi f) -> (h pi) f", pi=8
                        ),
                    )
                    RSQ = work.tile([32, 128], F32, tag="rsq")
                    nc.vector.reciprocal(out=RSQ[:], in_=DSQ[:])
                    nc.sync.dma_start(
                        drr[4 * t2 : 4 * t2 + 4, :].rearrange(
                            "h (pi f) -> (h pi) f", pi=8
                        ),
                        RSQ[:],
                    )
                    B = big.tile([128, SEQ], F32, tag=f"bc{t2}", name=f"bc{t2}")
                    for c2 in range(4):
                        h = 4 * t2 + c2
                        nc.sync.dma_start(
                            B[32 * c2 : 32 * c2 + 32, :],
                            drr[h : h + 1, :].to_broadcast([32, SEQ]),
                        )
                    Sd = big.tile([128, SEQ], F32R, tag=f"sc{t2}", name=f"sc{t2}")
                    nc.vector.tensor_mul(Sd[:], SCr[t2][:], B[:])
                    SC.append(Sd)


            # ---- stage D: output projection (fp32r) ----
            with tc.tile_pool(name="ps_o", bufs=2, space="PSUM") as ps_o:
                for ib in range(8):
                    po = ps_o.tile([128, EMB], F32, tag="po")
                    for ic in range(2):
                        for t2 in range(4):
                            nc.tensor.matmul(
                                po[:, 512 * ic : 512 * ic + 512],
                                SC[t2][:, 128 * ib : 128 * ib + 128],
                                WO[:, t2, 512 * ic : 512 * ic + 512],
                                start=(t2 == 0),
                                stop=(t2 == 3),
                            )
                    Y = work.tile([128, EMB], F32, tag="y")
                    nc.vector.tensor_copy(Y[:], po[:])
                    nc.sync.dma_start(y_d[128 * ib : 128 * ib + 128, :], Y[:])

    split_excess_waits(nc)
    return nc


_NC_CACHE = None


def _get_nc():
    global _NC_CACHE
    if _NC_CACHE is None:
        _NC_CACHE = build_nc()
    return _NC_CACHE


def kernel(x, Wq, Wk, Wv, Wo, bo):
    x = np.asarray(x, dtype=np.float32)
    Wq = np.asarray(Wq, dtype=np.float32)
    Wk = np.asarray(Wk, dtype=np.float32)
    Wv = np.asarray(Wv, dtype=np.float32)
    Wo = np.asarray(Wo, dtype=np.float32)
    bo = np.asarray(bo, dtype=np.float32)

    nc = _get_nc()
    in_maps = []
    for c in range(8):
        b, g = divmod(c, NG)
        cols = slice(GCOLS * g, GCOLS * g + GCOLS)
        # Wo rows for this group, padded to the ctx-psum row layout:
        # quad t2, col-group c2, row r<16 -> Wo[g*256 + (4*t2+c2)*16 + r]
        wo_aug = np.zeros((512, EMB), dtype=np.float32)
        wo_g = Wo[cols, :]
        for t2 in range(4):
            for c2 in range(4):
                h = (t2 // 2) * 8 + 2 * c2 + (t2 % 2)
                wo_aug[128 * t2 + 32 * c2 : 128 * t2 + 32 * c2 + 16, :] = wo_g[
                    16 * h : 16 * h + 16, :
                ]
        in_maps.append(
            {
                "xT": np.ascontiguousarray(x[b].T),
                "wq": np.ascontiguousarray(Wq[:, cols]),
                "wk": np.ascontiguousarray(Wk[:, cols]),
                "wv": np.ascontiguousarray(Wv[:, cols]),
                "wo": wo_aug,
            }
        )

    res = run_bass_kernel_spmd(nc, in_maps, core_ids=list(range(8)))
    out = np.zeros((BATCH, SEQ, EMB), dtype=np.float32)
    for c in range(8):
        b = c // NG
        out[b] += res.results[c]["y"]
    out += bo[None, None, :]
    return out



# revision 5
# speedup vs baseline: 1.1147x; 1.1147x over previous
"""Multi-head attention (axis-swapped variant) on 8 Trainium2 NeuronCores.

Reference semantics (EMB=1024): 64 effective heads of size 16 acting on the
d_head axis, causal softmax scaled by 1/sqrt(16), projections Wq/Wk/Wv,
output projection Wo + bo.

Sharding: core c = 4*b + g handles batch b and head-group g (16 heads =
256 contiguous projection columns). Each core returns a partial output
[1024, 1024]; the host sums the 4 group partials per batch and adds bo.

Per-core pipeline:
- bf16 Q/K/V projections (fp32 PSUM accumulate)
- Q/K evacuated to fp8e4m3 with columns pre-ordered (e,h); a DRAM
  roundtrip reshapes them to [8, qk, 2, 16, seq] so the score matmuls can
  run in fp8 DoubleRow mode (contraction 16 = 8 partitions x 2 k-tiles)
- softmax exp split across three engines: ACT true exp, DVE/Pool use the
  Schraudolph bit-trick (y = int16(x*A+B) bitcast to bf16)
- causal diag masking via post-exp multiply with a lower-tri constant
- ctx accumulated transposed: out [128 queries, 17] per head (16 v-dims +
  ones-column denominator) so the PE free-size stays tiny
- normalize, PE transpose, bf16 out-projection, direct PSUM->DRAM output
"""

import numpy as np
import ml_dtypes

import concourse.bass as bass
import concourse.mybir as mybir
import concourse.tile as tile
from concourse.bass_utils import run_bass_kernel_spmd

F32 = mybir.dt.float32
BF16 = mybir.dt.bfloat16
F8 = mybir.dt.float8e4
I16 = mybir.dt.int16
BF = ml_dtypes.bfloat16

EMB = 1024
SEQ = 1024
BATCH = 2
NG = 4            # head groups (cores per batch)
HPG = 16          # heads per group/core
DH = 16           # per-head feature size
GCOLS = HPG * DH  # 256 projection columns per core

DR = mybir.MatmulPerfMode.DoubleRow
MULT = mybir.AluOpType.mult
ADD = mybir.AluOpType.add
EXPF = mybir.ActivationFunctionType.Exp

# Schraudolph exp: bf16(bitcast_int16(s * A_S + B_S)) ~= exp(0.25 * s)
A_S = float(np.float32(0.25 * 128.0 / np.log(2.0)))
B_S = 16248.0

# engine assignment patterns (tuned against TimelineSim)
EXP_PATTERN = ("act", "dve", "act", "pool", "act", "dve", "act", "pool")
MASK_PATTERN = ("dve", "pool")
QK_EVAC = ("pool", "dve")  # per qk index


def split_excess_waits(nc, cap=1):
    """This container's walrus rejects instructions carrying more than a few
    semaphore waits (and bass's own model says one). Relocate excess waits
    onto preceding same-engine EventSemaphore instructions."""

    def fix_block(bb, dummy):
        insts = bb.instructions
        i = 0
        while i < len(insts):
            inst = insts[i]
            si = inst.sync_info
            waits = list(si.on_wait) if si is not None and si.on_wait else []
            if len(waits) > cap:
                eng = nc.engines[inst.engine]
                excess, keep = waits[:-cap], waits[-cap:]
                si.on_wait = keep
                pos = i
                for j in range(0, len(excess), cap):
                    chunk = excess[j : j + cap]
                    ev = eng.wait_ge(dummy, 1)
                    cur_list = nc.cur_bb.bb.instructions
                    assert cur_list[-1] is ev.ins
                    cur_list.pop()
                    ev.ins.sync_info.on_wait = chunk
                    insts.insert(pos, ev.ins)
                    pos += 1
                    i += 1
            i += 1

    with nc.semaphore("waitfix_dummy") as dummy:
        for f in nc.m.functions:
            for bb in f.blocks:
                fix_block(bb, dummy)


def _bcast(ap, dim, count):
    """Insert a stride-0 dim at position `dim` of an AP."""
    new_ap = list(ap.ap)
    new_ap.insert(dim, [0, count])
    return bass.AP(tensor=ap.tensor, offset=ap.offset, ap=new_ap)


def build_nc():
    nc = bass.Bass()
    xT_d = nc.declare_dram_parameter("xT", [EMB, SEQ], BF16, isOutput=False)
    wq_d = nc.declare_dram_parameter("wq", [EMB, GCOLS], BF16, isOutput=False)
    wk_d = nc.declare_dram_parameter("wk", [EMB, GCOLS], BF16, isOutput=False)
    wv_d = nc.declare_dram_parameter("wv", [EMB, GCOLS], BF16, isOutput=False)
    wo_d = nc.declare_dram_parameter("wo", [GCOLS, EMB], BF16, isOutput=False)
    cst_d = nc.declare_dram_parameter("cst", [128, 256], BF16, isOutput=False)
    y_d = nc.declare_dram_parameter("y", [SEQ, EMB], F32, isOutput=True)

    with tile.TileContext(nc) as tc:
        with (
            tc.tile_pool(name="big", bufs=1) as big,
            tc.tile_pool(name="att", bufs=16) as att,
            tc.tile_pool(name="work", bufs=4) as work,
            tc.tile_pool(name="dram", bufs=1, space="DRAM") as dram,
        ):
            # ---- input DMAs ----
            WQ = big.tile([128, 8, GCOLS], BF16)
            nc.sync.dma_start(WQ[:], wq_d[:].rearrange("(kb p) n -> p kb n", p=128))
            WK = big.tile([128, 8, GCOLS], BF16)
            nc.sync.dma_start(WK[:], wk_d[:].rearrange("(kb p) n -> p kb n", p=128))
            XT = big.tile([128, 8, SEQ], BF16)
            xT_r = xT_d[:].rearrange("(kb p) m -> p kb m", p=128)
            for mh in range(2):
                nc.sync.dma_start(
                    XT[:, :, 512 * mh : 512 * mh + 512],
                    xT_r[:, :, 512 * mh : 512 * mh + 512],
                )
            WV = big.tile([128, 8, GCOLS], BF16)
            nc.sync.dma_start(WV[:], wv_d[:].rearrange("(kb p) n -> p kb n", p=128))
            WO = big.tile([128, 2, EMB], BF16)
            nc.sync.dma_start(WO[:], wo_d[:].rearrange("(ch p) n -> p ch n", p=128))
            CST = big.tile([128, 256], BF16)
            nc.sync.dma_start(CST[:], cst_d[:])
            MASK = CST[:, 0:128]
            IDENT = CST[:, 128:256]

            QK8 = big.tile([128, 2, 2, SEQ], F8)       # (p=col, ct, qk, m)
            QKT8 = big.tile([8, 2, 2, HPG, SEQ], F8)   # (p8, qk, i, h, m)
            VA = big.tile([128, 8, HPG, 17], BF16)     # (p=key, kb, h, 16v+1)
            nc.gpsimd.memset(VA[:, :, :, 16:17], 1.0)
            ZL = big.tile([8, 2, 128], F8)
            nc.gpsimd.memset(ZL[:], 0.0)
            ZR = big.tile([8, 2, 272], F8)
            nc.gpsimd.memset(ZR[:], 0.0)
            CN = big.tile([128, 8, GCOLS], BF16)       # normalized ctx per qq
            qk8_d = dram.tile([2, 2, 128, SEQ], F8)    # (qk, ct, p, m)

            vec = {"dve": nc.vector, "pool": nc.gpsimd}

            # ---- Phase 1: Q/K proj (+fp8 roundtrip), V proj ----
            with tc.tile_pool(name="ps_p", bufs=3, space="PSUM") as ps_p:
                for mh in range(2):
                    for qki, Wt in enumerate((WQ, WK)):
                        for ct in range(2):
                            pq = ps_p.tile([128, 512], F32, tag="pp")
                            for kb in range(8):
                                nc.tensor.matmul(
                                    pq[:],
                                    Wt[:, kb, 128 * ct : 128 * ct + 128],
                                    XT[:, kb, 512 * mh : 512 * mh + 512],
                                    start=(kb == 0),
                                    stop=(kb == 7),
                                )
                            vec[QK_EVAC[qki]].tensor_copy(
                                QK8[:, ct, qki, 512 * mh : 512 * mh + 512], pq[:]
                            )
                        nc.sync.dma_start(
                            qk8_d[qki].rearrange("ct p m -> p ct m")[
                                :, :, 512 * mh : 512 * mh + 512
                            ],
                            QK8[:, :, qki, 512 * mh : 512 * mh + 512],
                        )
                        for i in range(2):
                            nc.sync.dma_start(
                                QKT8[:, qki, i, :, 512 * mh : 512 * mh + 512],
                                qk8_d[qki, i].rearrange("(p8 h) m -> p8 h m", p8=8)[
                                    :, :, 512 * mh : 512 * mh + 512
                                ],
                            )
                # V proj after both QK halves (keeps PE busy during roundtrip)
                for mt in range(8):
                    pv = ps_p.tile([128, GCOLS], F32, tag="pv")
                    for kb in range(8):
                        nc.tensor.matmul(
                            pv[:],
                            XT[:, kb, 128 * mt : 128 * mt + 128],
                            WV[:, kb, :],
                            start=(kb == 0),
                            stop=(kb == 7),
                        )
                    nc.vector.tensor_copy(
                        VA[:, mt, :, 0:16],
                        pv[:].rearrange("p (h e) -> p h e", e=DH),
                    )

            # ---- Phase 2: attention ----
            exp_i = 0
            mask_i = 0
            with (
                tc.tile_pool(name="ps_sw", bufs=2, space="PSUM") as ps_sw,
                tc.tile_pool(name="ps_c", bufs=4, space="PSUM") as ps_c,
            ):
                for ic in range(2):
                    c0 = 512 * ic
                    nkb = 4 * (ic + 1)
                    CTX = []
                    for qb in range(4):
                        t = ps_c.tile([128, 512], F32, tag="ctx", name=f"ctx{ic}{qb}")
                        nc.tensor.matmul(
                            t[:, 0:272],
                            ZL[:],
                            ZR[:],
                            start=True,
                            stop=False,
                            perf_mode=DR,
                            skip_group_check=True,
                        )
                        CTX.append(t)

                    def emit_ctx(kb, ATs, ic=ic, CTX=CTX):
                        for qb in range(max(0, kb - 4 * ic), 4):
                            for pr in range(8):
                                for ph in range(2):
                                    h = 2 * pr + ph
                                    nc.tensor.matmul(
                                        CTX[qb][:, 17 * h : 17 * h + 17],
                                        ATs[pr][:, ph, 128 * qb : 128 * qb + 128],
                                        VA[:, kb, h, :],
                                        start=False,
                                        stop=False,
                                        skip_group_check=True,
                                    )

                    prev = None
                    for kb in range(nkb):
                        lo = max(c0, 128 * kb)
                        j0 = lo - c0
                        ATs = []
                        for pr in range(8):
                            SW = ps_sw.tile([128, 2, 512], F32, tag="sw")
                            for ph in range(2):
                                h = 2 * pr + ph
                                nc.tensor.matmul(
                                    SW[:, ph, j0:512],
                                    QKT8[:, 1, :, h, 128 * kb : 128 * kb + 128],
                                    QKT8[:, 0, :, h, c0 + j0 : c0 + 512],
                                    start=True,
                                    stop=True,
                                    perf_mode=DR,
                                )
                            AT = att.tile([128, 2, 512], BF16, tag="at")
                            eng = EXP_PATTERN[exp_i % len(EXP_PATTERN)]
                            exp_i += 1
                            if eng == "act":
                                nc.scalar.activation(
                                    AT[:, :, j0:512], SW[:, :, j0:512], EXPF,
                                    scale=0.25,
                                )
                            else:
                                ATi = AT.bitcast(I16)
                                vec[eng].tensor_scalar(
                                    ATi[:, :, j0:512], SW[:, :, j0:512],
                                    A_S, B_S, MULT, ADD,
                                )
                            if 128 * kb >= c0:
                                meng = MASK_PATTERN[mask_i % len(MASK_PATTERN)]
                                mask_i += 1
                                vec[meng].tensor_tensor(
                                    AT[:, :, j0 : j0 + 128],
                                    AT[:, :, j0 : j0 + 128],
                                    _bcast(MASK, 1, 2),
                                    op=MULT,
                                )
                            ATs.append(AT)
                        if prev is not None:
                            emit_ctx(*prev)
                        prev = (kb, ATs)
                    emit_ctx(*prev)

                    for qb in range(4):
                        qq = 4 * ic + qb
                        ctx3 = CTX[qb][:, 0:272].rearrange("p (h e) -> p h e", e=17)
                        RG = work.tile([128, HPG], F32, tag="rg")
                        nc.vector.reciprocal(out=RG[:], in_=ctx3[:, :, 16])
                        nc.vector.tensor_tensor(
                            CN[:, qq, :].rearrange("p (h e) -> p h e", e=DH),
                            ctx3[:, :, 0:16],
                            _bcast(RG[:], 2, DH),
                            op=MULT,
                        )

            # ---- Phase 3: transpose + output projection ----
            ycp = [nc.scalar.copy, nc.vector.tensor_copy, nc.gpsimd.tensor_copy]
            with (
                tc.tile_pool(name="ps_t", bufs=2, space="PSUM") as ps_t,
                tc.tile_pool(name="ps_o", bufs=2, space="PSUM") as ps_o,
            ):
                for qq in range(8):
                    CT = work.tile([128, 2, 128], BF16, tag="ct")
                    for ch in range(2):
                        TP = ps_t.tile([128, 128], BF16, tag="tp")
                        nc.tensor.matmul(
                            TP[:],
                            CN[:, qq, 128 * ch : 128 * ch + 128],
                            IDENT,
                            is_transpose=True,
                        )
                        nc.vector.tensor_copy(CT[:, ch, :], TP[:])
                    PO = ps_o.tile([128, EMB], F32, tag="po")
                    for nh in range(2):
                        for ch in range(2):
                            nc.tensor.matmul(
                                PO[:, 512 * nh : 512 * nh + 512],
                                CT[:, ch, :],
                                WO[:, ch, 512 * nh : 512 * nh + 512],
                                start=(ch == 0),
                                stop=(ch == 1),
                            )
                    Y = work.tile([128, EMB], F32, tag="y")
                    for nh in range(2):
                        ycp[(2 * qq + nh) % 3](
                            Y[:, 512 * nh : 512 * nh + 512],
                            PO[:, 512 * nh : 512 * nh + 512],
                        )
                    nc.sync.dma_start(y_d[128 * qq : 128 * qq + 128, :], Y[:])

    split_excess_waits(nc)
    return nc


_NC_CACHE = None


def _get_nc():
    global _NC_CACHE
    if _NC_CACHE is None:
        _NC_CACHE = build_nc()
    return _NC_CACHE


# column permutation: device col j = 16*e + h  <-  module-local col 16*h + e
_PERM = [(j % 16) * 16 + j // 16 for j in range(GCOLS)]


def kernel(x, Wq, Wk, Wv, Wo, bo):
    x = np.asarray(x, dtype=np.float32)
    Wq = np.asarray(Wq, dtype=np.float32)
    Wk = np.asarray(Wk, dtype=np.float32)
    Wv = np.asarray(Wv, dtype=np.float32)
    Wo = np.asarray(Wo, dtype=np.float32)
    bo = np.asarray(bo, dtype=np.float32)

    cst = np.zeros((128, 256), dtype=BF)
    cst[:, 0:128] = np.triu(np.ones((128, 128), dtype=np.float32)).astype(BF)
    cst[:, 128:256] = np.eye(128, dtype=np.float32).astype(BF)

    nc = _get_nc()
    in_maps = []
    for c in range(8):
        b, g = divmod(c, NG)
        cols = slice(GCOLS * g, GCOLS * g + GCOLS)
        in_maps.append(
            {
                "xT": np.ascontiguousarray(x[b].T).astype(BF),
                "wq": np.ascontiguousarray(Wq[:, cols][:, _PERM]).astype(BF),
                "wk": np.ascontiguousarray(Wk[:, cols][:, _PERM]).astype(BF),
                "wv": np.ascontiguousarray(Wv[:, cols]).astype(BF),
                "wo": np.ascontiguousarray(Wo[cols, :]).astype(BF),
                "cst": cst,
            }
        )

    res = run_bass_kernel_spmd(nc, in_maps, core_ids=list(range(8)))
    out = np.zeros((BATCH, SEQ, EMB), dtype=np.float32)
    for c in range(8):
        b = c // NG
        out[b] += res.results[c]["y"]
    out += bo[None, None, :]
    return out


# revision 11
# speedup vs baseline: 1.2034x; 1.0796x over previous
"""Multi-head attention (axis-swapped variant) on 8 Trainium2 NeuronCores.

Reference semantics (EMB=1024): 64 effective heads of size 16 acting on the
d_head axis, causal softmax scaled by 1/sqrt(16), projections Wq/Wk/Wv,
output projection Wo + bo.

Sharding: core c = 4*b + g handles batch b and head-group g (16 heads =
256 contiguous projection columns). Each core returns a partial output
[1024, 1024]; the host sums the 4 group partials per batch and adds bo.

Per-core pipeline:
- bf16 Q/K/V projections (fp32 PSUM accumulate)
- Q/K evacuated to fp8e4m3 with columns pre-ordered (e,h); a DRAM
  roundtrip reshapes them to [8, qk, 2, 16, seq] so the score matmuls can
  run in fp8 DoubleRow mode (contraction 16 = 8 partitions x 2 k-tiles)
- causal diag masking as an extra fp8-DoubleRow matmul adding -120 above
  the diagonal (identity lhsT, precomputed mneg rhs) before the exp
- softmax exp split across three engines: ACT true exp, DVE/Pool use the
  Schraudolph bit-trick (y = int16(x*A+B) bitcast to bf16)
- ctx accumulated transposed: out [128 queries, 16] per head, denominators
  via separate free-size-1 matmuls against a ones column
- normalize, PE transpose, bf16 out-projection
"""

import numpy as np
import ml_dtypes

import concourse.bass as bass
import concourse.mybir as mybir
import concourse.tile as tile
from concourse.bass_utils import run_bass_kernel_spmd

F32 = mybir.dt.float32
BF16 = mybir.dt.bfloat16
F8 = mybir.dt.float8e4
I16 = mybir.dt.int16
BF = ml_dtypes.bfloat16
F8NP = ml_dtypes.float8_e4m3

EMB = 1024
SEQ = 1024
BATCH = 2
NG = 4            # head groups (cores per batch)
HPG = 16          # heads per group/core
DH = 16           # per-head feature size
GCOLS = HPG * DH  # 256 projection columns per core

DR = mybir.MatmulPerfMode.DoubleRow
MULT = mybir.AluOpType.mult
ADD = mybir.AluOpType.add
EXPF = mybir.ActivationFunctionType.Exp

MNEG = -120.0
# Schraudolph exp: bf16(bitcast_int16(s * A_S + B_S)) ~= exp(0.25 * s)
A_S = float(np.float32(0.25 * 128.0 / np.log(2.0)))
B_S = 16248.0

N_WARM = 10  # PE p-state warmup matmuls while input DMAs land


def split_excess_waits(nc, cap=1):
    """This container's walrus rejects instructions carrying more than a few
    semaphore waits (and bass's own model says one). Relocate excess waits
    onto preceding same-engine EventSemaphore instructions."""

    def fix_block(bb, dummy):
        insts = bb.instructions
        i = 0
        while i < len(insts):
            inst = insts[i]
            si = inst.sync_info
            waits = list(si.on_wait) if si is not None and si.on_wait else []
            if len(waits) > cap:
                eng = nc.engines[inst.engine]
                excess, keep = waits[:-cap], waits[-cap:]
                si.on_wait = keep
                pos = i
                for j in range(0, len(excess), cap):
                    chunk = excess[j : j + cap]
                    ev = eng.wait_ge(dummy, 1)
                    cur_list = nc.cur_bb.bb.instructions
                    assert cur_list[-1] is ev.ins
                    cur_list.pop()
                    ev.ins.sync_info.on_wait = chunk
                    insts.insert(pos, ev.ins)
                    pos += 1
                    i += 1
            i += 1

    with nc.semaphore("waitfix_dummy") as dummy:
        for f in nc.m.functions:
            for bb in f.blocks:
                fix_block(bb, dummy)


def _bcast(ap, dim, count):
    """Insert a stride-0 dim at position `dim` of an AP."""
    new_ap = list(ap.ap)
    new_ap.insert(dim, [0, count])
    return bass.AP(tensor=ap.tensor, offset=ap.offset, ap=new_ap)


class ExpSplit:
    """Greedy load-balancing of exp work across ACT / DVE / Pool."""

    def __init__(self, nc):
        self.nc = nc
        # preload with approximate non-exp duties (ns)
        self.load = {"act": 2500.0, "dve": 14000.0, "pool": 9000.0}
        self.cost = {
            "act": lambda r: r * 0.8333 + 370.0,
            "dve": lambda r: r * 1.0417 + 260.0,
            "pool": lambda r: r * 1.389 + 140.0,
        }

    def emit(self, at, ati, sw, rows):
        eng = min(self.load, key=lambda e: self.load[e] + self.cost[e](rows))
        self.load[eng] += self.cost[eng](rows)
        if eng == "act":
            self.nc.scalar.activation(at, sw, EXPF, scale=0.25)
        elif eng == "dve":
            self.nc.vector.tensor_scalar(ati, sw, A_S, B_S, MULT, ADD)
        else:
            self.nc.gpsimd.tensor_scalar(ati, sw, A_S, B_S, MULT, ADD)

    def add(self, eng, ns):
        self.load[eng] += ns


def build_nc():
    nc = bass.Bass()
    xT_d = nc.declare_dram_parameter("xT", [EMB, SEQ], BF16, isOutput=False)
    wq_d = nc.declare_dram_parameter("wq", [EMB, GCOLS], BF16, isOutput=False)
    wk_d = nc.declare_dram_parameter("wk", [EMB, GCOLS], BF16, isOutput=False)
    wv_d = nc.declare_dram_parameter("wv", [EMB, GCOLS], BF16, isOutput=False)
    wo_d = nc.declare_dram_parameter("wo", [GCOLS, EMB], BF16, isOutput=False)
    c8_d = nc.declare_dram_parameter("c8", [128, 2, 384], F8, isOutput=False)
    id_d = nc.declare_dram_parameter("idm", [128, 128], BF16, isOutput=False)
    y_d = nc.declare_dram_parameter("y", [SEQ, EMB], F32, isOutput=True)

    with tile.TileContext(nc) as tc:
        with (
            tc.tile_pool(name="big", bufs=1) as big,
            tc.tile_pool(name="att", bufs=16) as att,
            tc.tile_pool(name="work", bufs=4) as work,
            tc.tile_pool(name="dram", bufs=1, space="DRAM") as dram,
        ):
            # ---- input DMAs (order = SP queue order; no waits on any) ----
            xT_r = xT_d[:].rearrange("(kb p) m -> p kb m", p=128)
            XT = big.tile([128, 8, SEQ], BF16)
            WQ = big.tile([128, 8, GCOLS], BF16)
            WK = big.tile([128, 8, GCOLS], BF16)
            WV = big.tile([128, 8, GCOLS], BF16)
            WO = big.tile([128, 2, EMB], BF16)
            C8 = big.tile([128, 2, 384], F8)
            IDENT = big.tile([128, 128], BF16)

            def xchunk(ci, mh):
                sl = (slice(None), slice(2 * ci, 2 * ci + 2),
                      slice(512 * mh, 512 * mh + 512))
                nc.sync.dma_start(XT[sl], xT_r[sl])

            nc.sync.dma_start(WQ[:], wq_d[:].rearrange("(kb p) n -> p kb n", p=128))
            xchunk(0, 0)
            nc.sync.dma_start(WK[:], wk_d[:].rearrange("(kb p) n -> p kb n", p=128))
            xchunk(1, 0)
            xchunk(2, 0)
            xchunk(3, 0)
            nc.sync.dma_start(WV[:], wv_d[:].rearrange("(kb p) n -> p kb n", p=128))
            nc.sync.dma_start(C8[:], c8_d[:])
            for ci in range(4):
                xchunk(ci, 1)
            nc.sync.dma_start(WO[:], wo_d[:].rearrange("(ch p) n -> p ch n", p=128))
            nc.sync.dma_start(IDENT[:], id_d[:])

            MN2 = C8[:, :, 0:256]    # [p, i, (ph m)] additive -120 mask rhs
            ID2 = C8[:, :, 256:384]  # [p, i, j] identity pair lhsT

            QK8 = big.tile([128, 2, 2, SEQ], F8)       # (p=col, ct, qk, m)
            QKT8 = big.tile([8, 2, 2, HPG, SEQ], F8)   # (p8, qk, i, h, m)
            VA = big.tile([128, 8, HPG, DH], BF16)     # (p=key, kb, h, e)
            ONES = big.tile([128, 1], BF16)
            nc.gpsimd.memset(ONES[:], 1.0)
            ZL = big.tile([8, 2, 128], F8)
            nc.gpsimd.memset(ZL[:], 0.0)
            ZR = big.tile([8, 2, 512], F8)
            nc.gpsimd.memset(ZR[:], 0.0)
            CN = big.tile([128, 8, GCOLS], BF16)       # normalized ctx per qq
            qk8_d = dram.tile([2, 2, 128, SEQ], F8)    # (qk, ct, p, m)

            xs = ExpSplit(nc)
            evac = {0: nc.gpsimd, 1: nc.vector}  # qk evac: q->pool, k->dve

            def zero_mm(out_ap):
                nc.tensor.matmul(out_ap, ZL[:], ZR[:], start=True, stop=False,
                                 perf_mode=DR, skip_group_check=True)

            def proj_group(pq, Wt, qki, ct, mh):
                for kb in range(8):
                    nc.tensor.matmul(
                        pq[:],
                        Wt[:, kb, 128 * ct : 128 * ct + 128],
                        XT[:, kb, 512 * mh : 512 * mh + 512],
                        start=(kb == 0),
                        stop=(kb == 7),
                    )
                evac[qki].tensor_copy(
                    QK8[:, ct, qki, 512 * mh : 512 * mh + 512], pq[:]
                )

            def rt_dma(qki, mh):
                nc.sync.dma_start(
                    qk8_d[qki].rearrange("ct p m -> p ct m")[
                        :, :, 512 * mh : 512 * mh + 512
                    ],
                    QK8[:, :, qki, 512 * mh : 512 * mh + 512],
                )
                for i in range(2):
                    nc.sync.dma_start(
                        QKT8[:, qki, i, :, 512 * mh : 512 * mh + 512],
                        qk8_d[qki, i].rearrange("(p8 h) m -> p8 h m", p8=8)[
                            :, :, 512 * mh : 512 * mh + 512
                        ],
                    )

            def v_group(pool, mt, pv_bufs=1):
                pv = pool.tile([128, GCOLS], F32, tag="pv", name=f"pv{mt}",
                               bufs=pv_bufs)
                for kb in range(8):
                    nc.tensor.matmul(
                        pv[:],
                        XT[:, kb, 128 * mt : 128 * mt + 128],
                        WV[:, kb, :],
                        start=(kb == 0),
                        stop=(kb == 7),
                    )
                eng = nc.gpsimd if mt % 2 else nc.vector
                eng.tensor_copy(
                    VA[:, mt, :, :], pv[:].rearrange("p (h e) -> p h e", e=DH)
                )
                xs.add("pool" if mt % 2 else "dve", 420)

            # ---- P0: PE p-state warmup on zeros while DMAs land ----
            with tc.tile_pool(name="ps_w", bufs=1, space="PSUM") as ps_w:
                WARM = ps_w.tile([128, 512], F32, tag="warm")
                for _ in range(N_WARM):
                    nc.tensor.matmul(WARM[:], ZL[:], ZR[:], start=True, stop=True,
                                     perf_mode=DR, skip_group_check=True)

            # ---- P1a: Q/K proj mh0 (4 pq banks) + V mt0..3 ----
            with tc.tile_pool(name="ps_p", bufs=1, space="PSUM") as ps_p:
                pqs = {}
                for qki in range(2):
                    for ct in range(2):
                        pqs[(qki, ct)] = ps_p.tile(
                            [128, 512], F32, tag=f"pp{qki}{ct}", name=f"pq{qki}{ct}"
                        )
                for kb in range(8):
                    for qki, Wt in enumerate((WQ, WK)):
                        for ct in range(2):
                            nc.tensor.matmul(
                                pqs[(qki, ct)][:],
                                Wt[:, kb, 128 * ct : 128 * ct + 128],
                                XT[:, kb, 0:512],
                                start=(kb == 0),
                                stop=(kb == 7),
                            )
                for qki in range(2):
                    for ct in range(2):
                        evac[qki].tensor_copy(
                            QK8[:, ct, qki, 0:512], pqs[(qki, ct)][:]
                        )
                    rt_dma(qki, 0)
                xs.add("pool", 1500)
                xs.add("dve", 1500)
                for mt in range(4):
                    v_group(ps_p, mt, pv_bufs=2)

            # ---- attention over the two query halves ----
            def attention(ic, pool, filler):
                c0 = 512 * ic
                nkb = 4 * (ic + 1)
                DEN = pool.tile([128, 512], F32, tag="den", name=f"den{ic}")
                zero_mm(DEN[:])
                den_v = DEN[:, 0:64].rearrange("p (qb h) -> p qb h", h=HPG)
                CTXT = []
                for half in range(2):
                    t = pool.tile([128, 2, GCOLS], F32, tag="ctx", bufs=2,
                                  name=f"ctx{ic}{half}")
                    zero_mm(t[:])
                    CTXT.append(t)

                def ctx_of(qb):
                    return CTXT[qb // 2][:, qb % 2, :]

                def emit_ctx(kb, ATs):
                    for qb in range(max(0, kb - 4 * ic), 4):
                        for pr in range(8):
                            for ph in range(2):
                                h = 2 * pr + ph
                                lhsT = ATs[pr][:, ph, 128 * qb : 128 * qb + 128]
                                nc.tensor.matmul(
                                    ctx_of(qb)[:, DH * h : DH * h + DH],
                                    lhsT,
                                    VA[:, kb, h, :],
                                    start=False,
                                    stop=False,
                                    skip_group_check=True,
                                )
                                nc.tensor.matmul(
                                    den_v[:, qb, h : h + 1],
                                    lhsT,
                                    ONES[:],
                                    start=False,
                                    stop=False,
                                    skip_group_check=True,
                                )

                prev = None
                for kb in range(nkb):
                    lo = max(c0, 128 * kb)
                    j0 = lo - c0
                    diag = 128 * kb >= c0
                    ATs = []
                    for pr in range(8):
                        SW = pool.tile([128, 2, 512], F32, tag="sw", bufs=2,
                                       name="sw")
                        for ph in range(2):
                            h = 2 * pr + ph
                            nc.tensor.matmul(
                                SW[:, ph, j0:512],
                                QKT8[:, 1, :, h, 128 * kb : 128 * kb + 128],
                                QKT8[:, 0, :, h, c0 + j0 : c0 + 512],
                                start=True,
                                stop=not diag,
                                perf_mode=DR,
                                skip_group_check=True,
                            )
                        if diag:
                            nc.tensor.matmul(
                                SW[:, :, j0 : j0 + 128],
                                ID2,
                                MN2,
                                start=False,
                                stop=True,
                                perf_mode=DR,
                                skip_group_check=True,
                            )
                        AT = att.tile([128, 2, 512], BF16, tag="at", name="at")
                        xs.emit(
                            AT[:, :, j0:512],
                            AT.bitcast(I16)[:, :, j0:512],
                            SW[:, :, j0:512],
                            2 * (512 - j0),
                        )
                        ATs.append(AT)
                    if prev is not None:
                        emit_ctx(*prev)
                    prev = (kb, ATs)
                    if kb < len(filler):
                        filler[kb]()
                emit_ctx(*prev)
                for f in filler[nkb:]:
                    f()

                for qb in range(4):
                    qq = 4 * ic + qb
                    ctx3 = ctx_of(qb).rearrange("p (h e) -> p h e", e=DH)
                    RG = work.tile([128, HPG], F32, tag="rg", name="rg")
                    nc.vector.reciprocal(out=RG[:], in_=den_v[:, qb, :])
                    nc.vector.tensor_tensor(
                        CN[:, qq, :].rearrange("p (h e) -> p h e", e=DH),
                        ctx3,
                        _bcast(RG[:], 2, DH),
                        op=MULT,
                    )
                    xs.add("dve", 900)

            # P2a: ic0, with Q/K mh1 projection interleaved (1 spare bank)
            with tc.tile_pool(name="ps_a0", bufs=1, space="PSUM") as ps_a0:

                def mk_proj_filler(qki, ct):
                    def f():
                        pq = ps_a0.tile([128, 512], F32, tag="pq2", name="pq2")
                        proj_group(pq, (WQ, WK)[qki], qki, ct, 1)
                        if ct == 1:
                            rt_dma(qki, 1)
                        xs.add(("pool", "dve")[qki], 750)

                    return f

                attention(
                    0,
                    ps_a0,
                    [mk_proj_filler(0, 0), mk_proj_filler(0, 1),
                     mk_proj_filler(1, 0), mk_proj_filler(1, 1)],
                )

            # P2b: ic1, with V mt4..7 interleaved
            with tc.tile_pool(name="ps_a1", bufs=1, space="PSUM") as ps_a1:
                attention(
                    1,
                    ps_a1,
                    [lambda mt=mt: v_group(ps_a1, mt) for mt in range(4, 8)],
                )

            # ---- P3: transpose + output projection ----
            ycp = [nc.scalar.copy, nc.vector.tensor_copy, nc.gpsimd.tensor_copy]
            with (
                tc.tile_pool(name="ps_t", bufs=2, space="PSUM") as ps_t,
                tc.tile_pool(name="ps_o", bufs=2, space="PSUM") as ps_o,
            ):
                for qq in range(8):
                    CT = work.tile([128, 2, 128], BF16, tag="ct", name="ct")
                    for ch in range(2):
                        TP = ps_t.tile([128, 128], BF16, tag="tp", name="tp")
                        nc.tensor.matmul(
                            TP[:],
                            CN[:, qq, 128 * ch : 128 * ch + 128],
                            IDENT[:],
                            is_transpose=True,
                        )
                        nc.vector.tensor_copy(CT[:, ch, :], TP[:])
                    PO = ps_o.tile([128, EMB], F32, tag="po", name="po")
                    for nh in range(2):
                        for ch in range(2):
                            nc.tensor.matmul(
                                PO[:, 512 * nh : 512 * nh + 512],
                                CT[:, ch, :],
                                WO[:, ch, 512 * nh : 512 * nh + 512],
                                start=(ch == 0),
                                stop=(ch == 1),
                            )
                    Y = work.tile([128, EMB], F32, tag="y", name="y")
                    for nh in range(2):
                        ycp[(2 * qq + nh) % 3](
                            Y[:, 512 * nh : 512 * nh + 512],
                            PO[:, 512 * nh : 512 * nh + 512],
                        )
                    nc.sync.dma_start(y_d[128 * qq : 128 * qq + 128, :], Y[:])

    split_excess_waits(nc)
    return nc


_NC_CACHE = None


def _get_nc():
    global _NC_CACHE
    if _NC_CACHE is None:
        _NC_CACHE = build_nc()
    return _NC_CACHE


# column permutation: device col j = 16*e + h  <-  module-local col 16*h + e
_PERM = [(j % 16) * 16 + j // 16 for j in range(GCOLS)]


def _consts():
    c8 = np.zeros((128, 2, 384), dtype=F8NP)
    j = np.arange(128)[:, None]
    m = np.arange(128)[None, :]
    mneg = np.where(j > m, np.float32(MNEG), np.float32(0.0))
    c8[:, 0, 0:128] = mneg.astype(F8NP)
    c8[:, 0, 128:256] = mneg.astype(F8NP)
    c8[:, 0, 256:384] = np.eye(128, dtype=np.float32).astype(F8NP)
    idm = np.eye(128, dtype=np.float32).astype(BF)
    return c8, idm


def kernel(x, Wq, Wk, Wv, Wo, bo):
    x = np.asarray(x, dtype=np.float32)
    Wq = np.asarray(Wq, dtype=np.float32)
    Wk = np.asarray(Wk, dtype=np.float32)
    Wv = np.asarray(Wv, dtype=np.float32)
    Wo = np.asarray(Wo, dtype=np.float32)
    bo = np.asarray(bo, dtype=np.float32)

    c8, idm = _consts()
    nc = _get_nc()
    in_maps = []
    for c in range(8):
        b, g = divmod(c, NG)
        cols = slice(GCOLS * g, GCOLS * g + GCOLS)
        in_maps.append(
            {
                "xT": np.ascontiguousarray(x[b].T).astype(BF),
                "wq": np.ascontiguousarray(Wq[:, cols][:, _PERM]).astype(BF),
                "wk": np.ascontiguousarray(Wk[:, cols][:, _PERM]).astype(BF),
                "wv": np.ascontiguousarray(Wv[:, cols]).astype(BF),
                "wo": np.ascontiguousarray(Wo[cols, :]).astype(BF),
                "c8": c8,
                "idm": idm,
            }
        )

    res = run_bass_kernel_spmd(nc, in_maps, core_ids=list(range(8)))
    out = np.zeros((BATCH, SEQ, EMB), dtype=np.float32)
    for c in range(8):
        b = c // NG
        out[b] += res.results[c]["y"]
    out += bo[None, None, :]
    return out


# revision 13
# speedup vs baseline: 1.4204x; 1.1803x over previous
"""Multi-head attention (axis-swapped variant) on 8 Trainium2 NeuronCores.

Reference semantics (EMB=1024): 64 effective heads of size 16 acting on the
d_head axis, causal softmax scaled by 1/sqrt(16), projections Wq/Wk/Wv,
output projection Wo + bo.

Sharding: core c = 4*b + g handles batch b and head-group g (16 heads =
256 contiguous projection columns). Each core returns a partial output
[1024, 1024]; the host sums the 4 group partials per batch and adds bo.

Per-core pipeline:
- bf16 Q/K/V projections (fp32 PSUM accumulate)
- Q/K evacuated to fp8e4m3 with columns pre-ordered (e,h); a DRAM
  roundtrip (parallel q/k chains on the ACT/DVE DMA queues) reshapes them
  to [8, qk, 2, 16, 512]-per-half so score matmuls run in fp8 DoubleRow
  mode (contraction 16 = 8 partitions x 2 k-tiles)
- causal diag masking as an extra fp8-DoubleRow matmul adding -120 above
  the diagonal (identity lhsT, precomputed mneg rhs) before the exp
- softmax exp split across three engines: ACT true exp, DVE/Pool use the
  Schraudolph bit-trick (y = int16(x*A+B) bitcast to bf16); score tiles
  are single-PSUM-bank [128, 2, 256] with 4 bufs for pipeline depth
- ctx accumulated transposed: out [128 queries, 16] per head, denominators
  via separate free-size-1 matmuls against a ones column
- normalize, PE transpose, bf16 out-projection
"""

import numpy as np
import ml_dtypes

import concourse.bass as bass
import concourse.mybir as mybir
import concourse.tile as tile
from concourse.bass_utils import run_bass_kernel_spmd

F32 = mybir.dt.float32
BF16 = mybir.dt.bfloat16
F8 = mybir.dt.float8e4
I16 = mybir.dt.int16
BF = ml_dtypes.bfloat16
F8NP = ml_dtypes.float8_e4m3

EMB = 1024
SEQ = 1024
BATCH = 2
NG = 4            # head groups (cores per batch)
HPG = 16          # heads per group/core
DH = 16           # per-head feature size
GCOLS = HPG * DH  # 256 projection columns per core

DR = mybir.MatmulPerfMode.DoubleRow
MULT = mybir.AluOpType.mult
ADD = mybir.AluOpType.add
EXPF = mybir.ActivationFunctionType.Exp

MNEG = -120.0
# Schraudolph exp: bf16(bitcast_int16(s * A_S + B_S)) ~= exp(0.25 * s)
A_S = float(np.float32(0.25 * 128.0 / np.log(2.0)))
B_S = 16248.0

N_WARM = 10  # PE p-state warmup matmuls while input DMAs land
SW_BUFS = 4
AT_BUFS = 24


def split_excess_waits(nc, cap=1):
    """This container's walrus rejects instructions carrying more than a few
    semaphore waits (and bass's own model says one). Relocate excess waits
    onto preceding same-engine EventSemaphore instructions."""

    def fix_block(bb, dummy):
        insts = bb.instructions
        i = 0
        while i < len(insts):
            inst = insts[i]
            si = inst.sync_info
            waits = list(si.on_wait) if si is not None and si.on_wait else []
            if len(waits) > cap:
                eng = nc.engines[inst.engine]
                excess, keep = waits[:-cap], waits[-cap:]
                si.on_wait = keep
                pos = i
                for j in range(0, len(excess), cap):
                    chunk = excess[j : j + cap]
                    ev = eng.wait_ge(dummy, 1)
                    cur_list = nc.cur_bb.bb.instructions
                    assert cur_list[-1] is ev.ins
                    cur_list.pop()
                    ev.ins.sync_info.on_wait = chunk
                    insts.insert(pos, ev.ins)
                    pos += 1
                    i += 1
            i += 1

    with nc.semaphore("waitfix_dummy") as dummy:
        for f in nc.m.functions:
            for bb in f.blocks:
                fix_block(bb, dummy)


def _bcast(ap, dim, count):
    """Insert a stride-0 dim at position `dim` of an AP."""
    new_ap = list(ap.ap)
    new_ap.insert(dim, [0, count])
    return bass.AP(tensor=ap.tensor, offset=ap.offset, ap=new_ap)


class ExpSplit:
    """Greedy load-balancing of exp work across ACT / DVE / Pool."""

    def __init__(self, nc):
        self.nc = nc
        # preload with approximate non-exp duties (ns)
        self.load = {"act": 2500.0, "dve": 12000.0, "pool": 9000.0}
        self.cost = {
            "act": lambda r: r * 0.8333 + 370.0,
            "dve": lambda r: r * 1.0417 + 260.0,
            "pool": lambda r: r * 1.389 + 140.0,
        }

    def emit(self, at, ati, sw, rows):
        eng = min(self.load, key=lambda e: self.load[e] + self.cost[e](rows))
        self.load[eng] += self.cost[eng](rows)
        if eng == "act":
            self.nc.scalar.activation(at, sw, EXPF, scale=0.25)
        elif eng == "dve":
            self.nc.vector.tensor_scalar(ati, sw, A_S, B_S, MULT, ADD)
        else:
            self.nc.gpsimd.tensor_scalar(ati, sw, A_S, B_S, MULT, ADD)

    def add(self, eng, ns):
        self.load[eng] += ns


def build_nc():
    nc = bass.Bass()
    xT_d = nc.declare_dram_parameter("xT", [EMB, SEQ], BF16, isOutput=False)
    wq_d = nc.declare_dram_parameter("wq", [EMB, GCOLS], BF16, isOutput=False)
    wk_d = nc.declare_dram_parameter("wk", [EMB, GCOLS], BF16, isOutput=False)
    wv_d = nc.declare_dram_parameter("wv", [EMB, GCOLS], BF16, isOutput=False)
    wo_d = nc.declare_dram_parameter("wo", [GCOLS, EMB], BF16, isOutput=False)
    c8_d = nc.declare_dram_parameter("c8", [128, 2, 384], F8, isOutput=False)
    id_d = nc.declare_dram_parameter("idm", [128, 128], BF16, isOutput=False)
    y_d = nc.declare_dram_parameter("y", [SEQ, EMB], F32, isOutput=True)

    with tile.TileContext(nc) as tc:
        with (
            tc.tile_pool(name="big", bufs=1) as big,
            tc.tile_pool(name="att", bufs=AT_BUFS) as att,
            tc.tile_pool(name="work", bufs=4) as work,
            tc.tile_pool(name="dram", bufs=1, space="DRAM") as dram,
        ):
            # ---- input DMAs (order = SP queue order; no waits on any) ----
            xT_r = xT_d[:].rearrange("(kb p) m -> p kb m", p=128)
            XT = big.tile([128, 8, SEQ], BF16)
            WQ = big.tile([128, 8, GCOLS], BF16)
            WK = big.tile([128, 8, GCOLS], BF16)
            WV = big.tile([128, 8, GCOLS], BF16)
            WO = big.tile([128, 2, EMB], BF16)
            C8 = big.tile([128, 2, 384], F8)
            IDENT = big.tile([128, 128], BF16)

            def xchunk(ci, mh):
                sl = (slice(None), slice(2 * ci, 2 * ci + 2),
                      slice(512 * mh, 512 * mh + 512))
                nc.sync.dma_start(XT[sl], xT_r[sl])

            nc.sync.dma_start(WQ[:], wq_d[:].rearrange("(kb p) n -> p kb n", p=128))
            xchunk(0, 0)
            nc.sync.dma_start(WK[:], wk_d[:].rearrange("(kb p) n -> p kb n", p=128))
            xchunk(1, 0)
            xchunk(2, 0)
            xchunk(3, 0)
            nc.sync.dma_start(WV[:], wv_d[:].rearrange("(kb p) n -> p kb n", p=128))
            nc.sync.dma_start(C8[:], c8_d[:])
            for ci in range(4):
                xchunk(ci, 1)
            nc.sync.dma_start(WO[:], wo_d[:].rearrange("(ch p) n -> p ch n", p=128))
            nc.sync.dma_start(IDENT[:], id_d[:])

            MN2 = C8[:, :, 0:256]    # [p, i, (ph m)] additive -120 mask rhs
            ID2 = C8[:, :, 256:384]  # [p, i, j] identity pair lhsT

            QK8 = big.tile([128, 2, 2, SEQ], F8)       # (p=col, ct, qk, m)
            # per m-half fp8 score operands: (p8, qk, i, h, m)
            QKT8h = [
                big.tile([8, 2, 2, HPG, 512], F8, name=f"qkt8h{mh}")
                for mh in range(2)
            ]
            VA = big.tile([128, 8, HPG, DH], BF16)     # (p=key, kb, h, e)
            ONES = big.tile([128, 1], BF16)
            nc.gpsimd.memset(ONES[:], 1.0)
            ZL = big.tile([8, 2, 128], F8)
            nc.gpsimd.memset(ZL[:], 0.0)
            ZR = big.tile([8, 2, 512], F8)
            nc.gpsimd.memset(ZR[:], 0.0)
            CN = big.tile([128, 8, GCOLS], BF16)       # normalized ctx per qq
            qk8_d = dram.tile([2, 2, 2, 128, 512], F8)  # (mh, qk, ct, p, m)

            xs = ExpSplit(nc)
            evac = {0: nc.gpsimd, 1: nc.vector}   # qk evac: q->pool, k->dve
            rtq = {0: nc.scalar, 1: nc.sync}      # roundtrip DMA queues

            def zero_mm(out_ap):
                nc.tensor.matmul(out_ap, ZL[:], ZR[:], start=True, stop=False,
                                 perf_mode=DR, skip_group_check=True)

            def proj_group(pq, Wt, qki, ct, mh):
                for kb in range(8):
                    nc.tensor.matmul(
                        pq[:],
                        Wt[:, kb, 128 * ct : 128 * ct + 128],
                        XT[:, kb, 512 * mh : 512 * mh + 512],
                        start=(kb == 0),
                        stop=(kb == 7),
                    )
                evac[qki].tensor_copy(
                    QK8[:, ct, qki, 512 * mh : 512 * mh + 512], pq[:]
                )

            def rt_dma(qki, mh):
                eng = rtq[qki]
                eng.dma_start(
                    qk8_d[mh, qki].rearrange("ct p m -> p ct m"),
                    QK8[:, :, qki, 512 * mh : 512 * mh + 512],
                )
                eng.dma_start(
                    QKT8h[mh][:, qki],
                    qk8_d[mh, qki].rearrange("i (p8 h) m -> p8 i h m", p8=8),
                )

            def v_group(pool, mt, pv_bufs=1):
                pv = pool.tile([128, GCOLS], F32, tag="pv", name=f"pv{mt}",
                               bufs=pv_bufs)
                for kb in range(8):
                    nc.tensor.matmul(
                        pv[:],
                        XT[:, kb, 128 * mt : 128 * mt + 128],
                        WV[:, kb, :],
                        start=(kb == 0),
                        stop=(kb == 7),
                    )
                eng = nc.gpsimd if mt % 2 else nc.vector
                eng.tensor_copy(
                    VA[:, mt, :, :], pv[:].rearrange("p (h e) -> p h e", e=DH)
                )
                xs.add("pool" if mt % 2 else "dve", 420)

            # ---- P0: PE p-state warmup on zeros while DMAs land ----
            with tc.tile_pool(name="ps_w", bufs=1, space="PSUM") as ps_w:
                WARM = ps_w.tile([128, 512], F32, tag="warm")
                for _ in range(N_WARM):
                    nc.tensor.matmul(WARM[:], ZL[:], ZR[:], start=True, stop=True,
                                     perf_mode=DR, skip_group_check=True)

            # ---- P1a: Q/K proj mh0 (4 pq banks) + V mt0..3 ----
            with tc.tile_pool(name="ps_p", bufs=1, space="PSUM") as ps_p:
                pqs = {}
                for qki in range(2):
                    for ct in range(2):
                        pqs[(qki, ct)] = ps_p.tile(
                            [128, 512], F32, tag=f"pp{qki}{ct}", name=f"pq{qki}{ct}"
                        )
                for kb in range(8):
                    for qki, Wt in enumerate((WQ, WK)):
                        for ct in range(2):
                            nc.tensor.matmul(
                                pqs[(qki, ct)][:],
                                Wt[:, kb, 128 * ct : 128 * ct + 128],
                                XT[:, kb, 0:512],
                                start=(kb == 0),
                                stop=(kb == 7),
                            )
                for qki in range(2):
                    for ct in range(2):
                        evac[qki].tensor_copy(
                            QK8[:, ct, qki, 0:512], pqs[(qki, ct)][:]
                        )
                    rt_dma(qki, 0)
                xs.add("pool", 1500)
                xs.add("dve", 1500)
                for mt in range(4):
                    v_group(ps_p, mt, pv_bufs=2)

            # ---- attention over the two query halves ----
            def attention(ic, pool, filler):
                c0 = 512 * ic
                nkb = 4 * (ic + 1)
                DEN = pool.tile([128, 512], F32, tag="den", name=f"den{ic}")
                zero_mm(DEN[:])
                den_v = DEN[:, 0:64].rearrange("p (qb h) -> p qb h", h=HPG)
                CTXT = []
                for half in range(2):
                    t = pool.tile([128, 2, GCOLS], F32, tag="ctx", bufs=2,
                                  name=f"ctx{ic}{half}")
                    zero_mm(t[:])
                    CTXT.append(t)

                def ctx_of(qb):
                    return CTXT[qb // 2][:, qb % 2, :]

                def emit_ctx(kb, ATs):
                    for qb in range(max(0, kb - 4 * ic), 4):
                        qh, offc = qb // 2, 128 * (qb % 2)
                        for pr in range(8):
                            for ph in range(2):
                                h = 2 * pr + ph
                                lhsT = ATs[(pr, qh)][:, ph, offc : offc + 128]
                                nc.tensor.matmul(
                                    ctx_of(qb)[:, DH * h : DH * h + DH],
                                    lhsT,
                                    VA[:, kb, h, :],
                                    start=False,
                                    stop=False,
                                    skip_group_check=True,
                                )
                                nc.tensor.matmul(
                                    den_v[:, qb, h : h + 1],
                                    lhsT,
                                    ONES[:],
                                    start=False,
                                    stop=False,
                                    skip_group_check=True,
                                )

                prev = None
                for kb in range(nkb):
                    mhk, kbl = divmod(kb, 4)
                    lo = max(c0, 128 * kb)
                    j0 = lo - c0
                    diag = 128 * kb >= c0
                    qh_d = j0 // 256
                    ATs = {}
                    for pr in range(8):
                        for qh in range(qh_d, 2):
                            off = max(j0 - 256 * qh, 0)
                            SW = pool.tile([128, 2, 256], F32, tag="sw",
                                           bufs=SW_BUFS, name="sw")
                            for ph in range(2):
                                h = 2 * pr + ph
                                nc.tensor.matmul(
                                    SW[:, ph, off:256],
                                    QKT8h[mhk][:, 1, :, h,
                                               128 * kbl : 128 * kbl + 128],
                                    QKT8h[ic][:, 0, :, h,
                                              256 * qh + off : 256 * (qh + 1)],
                                    start=True,
                                    stop=not (diag and qh == qh_d),
                                    perf_mode=DR,
                                    skip_group_check=True,
                                )
                            if diag and qh == qh_d:
                                nc.tensor.matmul(
                                    SW[:, :, off : off + 128],
                                    ID2,
                                    MN2,
                                    start=False,
                                    stop=True,
                                    perf_mode=DR,
                                    skip_group_check=True,
                                )
                            AT = att.tile([128, 2, 256], BF16, tag="at",
                                          name="at")
                            xs.emit(
                                AT[:, :, off:256],
                                AT.bitcast(I16)[:, :, off:256],
                                SW[:, :, off:256],
                                2 * (256 - off),
                            )
                            ATs[(pr, qh)] = AT
                    if prev is not None:
                        emit_ctx(*prev)
                    prev = (kb, ATs)
                    if kb < len(filler):
                        filler[kb]()
                emit_ctx(*prev)
                for f in filler[nkb:]:
                    f()

                for qb in range(4):
                    qq = 4 * ic + qb
                    ctx3 = ctx_of(qb).rearrange("p (h e) -> p h e", e=DH)
                    RG = work.tile([128, HPG], F32, tag="rg", name="rg")
                    nc.vector.reciprocal(out=RG[:], in_=den_v[:, qb, :])
                    nc.vector.tensor_tensor(
                        CN[:, qq, :].rearrange("p (h e) -> p h e", e=DH),
                        ctx3,
                        _bcast(RG[:], 2, DH),
                        op=MULT,
                    )
                    xs.add("dve", 900)

            # P2a: ic0, with Q/K mh1 projection interleaved (1 spare bank)
            with tc.tile_pool(name="ps_a0", bufs=1, space="PSUM") as ps_a0:

                def mk_proj_filler(qki, ct):
                    def f():
                        pq = ps_a0.tile([128, 512], F32, tag="pq2", name="pq2")
                        proj_group(pq, (WQ, WK)[qki], qki, ct, 1)
                        if ct == 1:
                            rt_dma(qki, 1)
                        xs.add(("pool", "dve")[qki], 750)

                    return f

                attention(
                    0,
                    ps_a0,
                    [mk_proj_filler(0, 0), mk_proj_filler(0, 1),
                     mk_proj_filler(1, 0), mk_proj_filler(1, 1)],
                )

            # P2b: ic1, with V mt4..7 interleaved
            with tc.tile_pool(name="ps_a1", bufs=1, space="PSUM") as ps_a1:
                attention(
                    1,
                    ps_a1,
                    [lambda mt=mt: v_group(ps_a1, mt) for mt in range(4, 8)],
                )

            # ---- P3: transpose + output projection ----
            ycp = [nc.scalar.copy, nc.vector.tensor_copy, nc.gpsimd.tensor_copy]
            with (
                tc.tile_pool(name="ps_t", bufs=2, space="PSUM") as ps_t,
                tc.tile_pool(name="ps_o", bufs=2, space="PSUM") as ps_o,
            ):
                for qq in range(8):
                    CT = work.tile([128, 2, 128], BF16, tag="ct", name="ct")
                    for ch in range(2):
                        TP = ps_t.tile([128, 128], BF16, tag="tp", name="tp")
                        nc.tensor.matmul(
                            TP[:],
                            CN[:, qq, 128 * ch : 128 * ch + 128],
                            IDENT[:],
                            is_transpose=True,
                        )
                        nc.vector.tensor_copy(CT[:, ch, :], TP[:])
                    PO = ps_o.tile([128, EMB], F32, tag="po", name="po")
                    for nh in range(2):
                        for ch in range(2):
                            nc.tensor.matmul(
                                PO[:, 512 * nh : 512 * nh + 512],
                                CT[:, ch, :],
                                WO[:, ch, 512 * nh : 512 * nh + 512],
                                start=(ch == 0),
                                stop=(ch == 1),
                            )
                    Y = work.tile([128, EMB], F32, tag="y", name="y")
                    for nh in range(2):
                        ycp[(2 * qq + nh) % 3](
                            Y[:, 512 * nh : 512 * nh + 512],
                            PO[:, 512 * nh : 512 * nh + 512],
                        )
                    nc.sync.dma_start(y_d[128 * qq : 128 * qq + 128, :], Y[:])

    split_excess_waits(nc)
    return nc


_NC_CACHE = None


def _get_nc():
    global _NC_CACHE
    if _NC_CACHE is None:
        _NC_CACHE = build_nc()
    return _NC_CACHE


# column permutation: device col j = 16*e + h  <-  module-local col 16*h + e
_PERM = [(j % 16) * 16 + j // 16 for j in range(GCOLS)]


def _consts():
    c8 = np.zeros((128, 2, 384), dtype=F8NP)
    j = np.arange(128)[:, None]
    m = np.arange(128)[None, :]
    mneg = np.where(j > m, np.float32(MNEG), np.float32(0.0))
    c8[:, 0, 0:128] = mneg.astype(F8NP)
    c8[:, 0, 128:256] = mneg.astype(F8NP)
    c8[:, 0, 256:384] = np.eye(128, dtype=np.float32).astype(F8NP)
    idm = np.eye(128, dtype=np.float32).astype(BF)
    return c8, idm


def kernel(x, Wq, Wk, Wv, Wo, bo):
    x = np.asarray(x, dtype=np.float32)
    Wq = np.asarray(Wq, dtype=np.float32)
    Wk = np.asarray(Wk, dtype=np.float32)
    Wv = np.asarray(Wv, dtype=np.float32)
    Wo = np.asarray(Wo, dtype=np.float32)
    bo = np.asarray(bo, dtype=np.float32)

    c8, idm = _consts()
    nc = _get_nc()
    in_maps = []
    for c in range(8):
        b, g = divmod(c, NG)
        cols = slice(GCOLS * g, GCOLS * g + GCOLS)
        in_maps.append(
            {
                "xT": np.ascontiguousarray(x[b].T).astype(BF),
                "wq": np.ascontiguousarray(Wq[:, cols][:, _PERM]).astype(BF),
                "wk": np.ascontiguousarray(Wk[:, cols][:, _PERM]).astype(BF),
                "wv": np.ascontiguousarray(Wv[:, cols]).astype(BF),
                "wo": np.ascontiguousarray(Wo[cols, :]).astype(BF),
                "c8": c8,
                "idm": idm,
            }
        )

    res = run_bass_kernel_spmd(nc, in_maps, core_ids=list(range(8)))
    out = np.zeros((BATCH, SEQ, EMB), dtype=np.float32)
    for c in range(8):
        b = c // NG
        out[b] += res.results[c]["y"]
    out += bo[None, None, :]
    return out


# revision 21
# speedup vs baseline: 1.4768x; 1.0397x over previous
"""Multi-head attention (axis-swapped variant) on 8 Trainium2 NeuronCores.

Reference semantics (EMB=1024): 64 effective heads of size 16 acting on the
d_head axis, causal softmax scaled by 1/sqrt(16), projections Wq/Wk/Wv,
output projection Wo + bo.

Sharding: core c = 4*b + g handles batch b and head-group g (16 heads =
256 contiguous projection columns). Each core returns a partial output
[1024, 1024]; the host sums the 4 group partials per batch and adds bo.

Per-core pipeline:
- bf16 Q/K/V projections (fp32 PSUM accumulate)
- Q/K evacuated to fp8e4m3 with columns pre-ordered (e,h); a DRAM
  roundtrip (parallel q/k chains on the ACT/DVE DMA queues) reshapes them
  to [8, qk, 2, 16, 512]-per-half so score matmuls run in fp8 DoubleRow
  mode (contraction 16 = 8 partitions x 2 k-tiles)
- causal diag masking as an extra fp8-DoubleRow matmul adding -120 above
  the diagonal (identity lhsT, precomputed mneg rhs) before the exp
- softmax exp split across three engines: ACT true exp, DVE/Pool use the
  Schraudolph bit-trick (y = int16(x*A+B) bitcast to bf16); score tiles
  are single-PSUM-bank [128, 2, 256] with 4 bufs for pipeline depth
- ctx accumulated transposed: out [128 queries, 16] per head, denominators
  via separate free-size-1 matmuls against a ones column
- normalize, PE transpose, bf16 out-projection
"""

import numpy as np
import ml_dtypes

import concourse.bass as bass
import concourse.mybir as mybir
import concourse.tile as tile
from concourse.bass_utils import run_bass_kernel_spmd

F32 = mybir.dt.float32
BF16 = mybir.dt.bfloat16
F8 = mybir.dt.float8e4
I16 = mybir.dt.int16
BF = ml_dtypes.bfloat16
F8NP = ml_dtypes.float8_e4m3

EMB = 1024
SEQ = 1024
BATCH = 2
NG = 4            # head groups (cores per batch)
HPG = 16          # heads per group/core
DH = 16           # per-head feature size
GCOLS = HPG * DH  # 256 projection columns per core

DR = mybir.MatmulPerfMode.DoubleRow
MULT = mybir.AluOpType.mult
ADD = mybir.AluOpType.add
EXPF = mybir.ActivationFunctionType.Exp

MNEG = -120.0
# Schraudolph exp: bf16(bitcast_int16(s * A_S + B_S)) ~= exp(0.25 * s)
A_S = float(np.float32(0.25 * 128.0 / np.log(2.0)))
B_S = 16248.0

N_WARM = 10  # PE p-state warmup matmuls while input DMAs land
SW_BUFS = 5
AT_BUFS = 24


def split_excess_waits(nc, cap=1):
    """This container's walrus rejects instructions carrying more than a few
    semaphore waits (and bass's own model says one). Relocate excess waits
    onto preceding same-engine EventSemaphore instructions."""

    def fix_block(bb, dummy):
        insts = bb.instructions
        i = 0
        while i < len(insts):
            inst = insts[i]
            si = inst.sync_info
            waits = list(si.on_wait) if si is not None and si.on_wait else []
            if len(waits) > cap:
                eng = nc.engines[inst.engine]
                excess, keep = waits[:-cap], waits[-cap:]
                si.on_wait = keep
                pos = i
                for j in range(0, len(excess), cap):
                    chunk = excess[j : j + cap]
                    ev = eng.wait_ge(dummy, 1)
                    cur_list = nc.cur_bb.bb.instructions
                    assert cur_list[-1] is ev.ins
                    cur_list.pop()
                    ev.ins.sync_info.on_wait = chunk
                    insts.insert(pos, ev.ins)
                    pos += 1
                    i += 1
            i += 1

    with nc.semaphore("waitfix_dummy") as dummy:
        for f in nc.m.functions:
            for bb in f.blocks:
                fix_block(bb, dummy)


def _bcast(ap, dim, count):
    """Insert a stride-0 dim at position `dim` of an AP."""
    new_ap = list(ap.ap)
    new_ap.insert(dim, [0, count])
    return bass.AP(tensor=ap.tensor, offset=ap.offset, ap=new_ap)


class ExpSplit:
    """Greedy load-balancing of exp work across ACT / DVE / Pool."""

    def __init__(self, nc):
        self.nc = nc
        # preload with approximate non-exp duties (ns)
        self.load = {"act": 2500.0, "dve": 12000.0, "pool": 9000.0}
        self.cost = {
            "act": lambda r: r * 0.8333 + 370.0,
            "dve": lambda r: r * 1.0417 + 260.0,
            "pool": lambda r: r * 1.389 + 140.0,
        }

    def emit(self, at, ati, sw, rows):
        eng = min(self.load, key=lambda e: self.load[e] + self.cost[e](rows))
        self.load[eng] += self.cost[eng](rows)
        if eng == "act":
            self.nc.scalar.activation(at, sw, EXPF, scale=0.25)
        elif eng == "dve":
            self.nc.vector.tensor_scalar(ati, sw, A_S, B_S, MULT, ADD)
        else:
            self.nc.gpsimd.tensor_scalar(ati, sw, A_S, B_S, MULT, ADD)

    def add(self, eng, ns):
        self.load[eng] += ns


def build_nc():
    nc = bass.Bass()
    xT_d = nc.declare_dram_parameter("xT", [EMB, SEQ], BF16, isOutput=False)
    wq_d = nc.declare_dram_parameter("wq", [EMB, GCOLS], BF16, isOutput=False)
    wk_d = nc.declare_dram_parameter("wk", [EMB, GCOLS], BF16, isOutput=False)
    wv_d = nc.declare_dram_parameter("wv", [EMB, GCOLS], BF16, isOutput=False)
    wo_d = nc.declare_dram_parameter("wo", [GCOLS, EMB], BF16, isOutput=False)
    c8_d = nc.declare_dram_parameter("c8", [128, 2, 384], F8, isOutput=False)
    id_d = nc.declare_dram_parameter("idm", [128, 128], BF16, isOutput=False)
    y_d = nc.declare_dram_parameter("y", [SEQ, EMB], F32, isOutput=True)

    with tile.TileContext(nc) as tc:
        with (
            tc.tile_pool(name="big", bufs=1) as big,
            tc.tile_pool(name="att", bufs=AT_BUFS) as att,
            tc.tile_pool(name="work", bufs=4) as work,
            tc.tile_pool(name="dram", bufs=1, space="DRAM") as dram,
        ):
            # ---- input DMAs (order = SP queue order; no waits on any) ----
            xT_r = xT_d[:].rearrange("(kb p) m -> p kb m", p=128)
            XT = big.tile([128, 8, SEQ], BF16)
            WQ = big.tile([128, 8, GCOLS], BF16)
            WK = big.tile([128, 8, GCOLS], BF16)
            WV = big.tile([128, 8, GCOLS], BF16)
            WO = big.tile([128, 2, EMB], BF16)
            C8 = big.tile([128, 2, 384], F8)
            IDENT = big.tile([128, 128], BF16)

            def xchunk(ci):
                sl = (slice(None), slice(2 * ci, 2 * ci + 2), slice(None))
                nc.sync.dma_start(XT[sl], xT_r[sl])

            nc.sync.dma_start(WQ[:], wq_d[:].rearrange("(kb p) n -> p kb n", p=128))
            xchunk(0)
            nc.sync.dma_start(WK[:], wk_d[:].rearrange("(kb p) n -> p kb n", p=128))
            xchunk(1)
            xchunk(2)
            xchunk(3)
            nc.sync.dma_start(WV[:], wv_d[:].rearrange("(kb p) n -> p kb n", p=128))
            nc.sync.dma_start(C8[:], c8_d[:])
            nc.sync.dma_start(WO[:], wo_d[:].rearrange("(ch p) n -> p ch n", p=128))
            nc.sync.dma_start(IDENT[:], id_d[:])

            MN2 = C8[:, :, 0:256]    # [p, i, (ph m)] additive -120 mask rhs
            ID2 = C8[:, :, 256:384]  # [p, i, j] identity pair lhsT

            QK8 = big.tile([128, 2, 2, SEQ], F8)       # (p=col, ct, qk, m)
            # per m-half fp8 score operands: (p8, qk, i, h, m)
            QKT8h = [
                big.tile([8, 2, 2, HPG, 512], F8, name=f"qkt8h{mh}")
                for mh in range(2)
            ]
            VA = big.tile([128, 8, HPG, DH], BF16)     # (p=key, kb, h, e)
            ONES = big.tile([128, 1], BF16)
            nc.gpsimd.memset(ONES[:], 1.0)
            ZL = big.tile([8, 2, 128], F8)
            nc.gpsimd.memset(ZL[:], 0.0)
            ZR = big.tile([8, 2, 512], F8)
            nc.gpsimd.memset(ZR[:], 0.0)
            CN = big.tile([128, 8, GCOLS], BF16)       # normalized ctx per qq
            qk8_d = dram.tile([2, 2, 2, 128, 512], F8)  # (mh, qk, ct, p, m)

            xs = ExpSplit(nc)
            evac = {0: nc.gpsimd, 1: nc.vector}   # qk evac: q->pool, k->dve

            def zero_mm(out_ap):
                nc.tensor.matmul(out_ap, ZL[:], ZR[:], start=True, stop=False,
                                 perf_mode=DR, skip_group_check=True)

            def rt_dma(qki, mh):
                nc.sync.dma_start(
                    qk8_d[mh, qki].rearrange("ct p m -> p ct m"),
                    QK8[:, :, qki, 512 * mh : 512 * mh + 512],
                )
                nc.sync.dma_start(
                    QKT8h[mh][:, qki],
                    qk8_d[mh, qki].rearrange("i (p8 h) m -> p8 i h m", p8=8),
                )

            def v_group(pool, mt, pv_bufs=1):
                pv = pool.tile([128, GCOLS], F32, tag="pv", name=f"pv{mt}",
                               bufs=pv_bufs)
                for kb in range(8):
                    nc.tensor.matmul(
                        pv[:],
                        XT[:, kb, 128 * mt : 128 * mt + 128],
                        WV[:, kb, :],
                        start=(kb == 0),
                        stop=(kb == 7),
                    )
                eng = nc.gpsimd if mt % 2 else nc.vector
                eng.tensor_copy(
                    VA[:, mt, :, :], pv[:].rearrange("p (h e) -> p h e", e=DH)
                )
                xs.add("pool" if mt % 2 else "dve", 420)

            # ---- P0: PE p-state warmup on zeros while DMAs land ----
            with tc.tile_pool(name="ps_w", bufs=1, space="PSUM") as ps_w:
                WARM = ps_w.tile([128, 512], F32, tag="warm")
                for _ in range(N_WARM):
                    nc.tensor.matmul(WARM[:], ZL[:], ZR[:], start=True, stop=True,
                                     perf_mode=DR, skip_group_check=True)

            # ---- P1a: Q/K proj both halves (8 pq banks), then V mt0..7 ----
            with tc.tile_pool(name="ps_p", bufs=1, space="PSUM") as ps_p:
                pqs = {}
                for qki in range(2):
                    for ct in range(2):
                        for mh in range(2):
                            pqs[(qki, ct, mh)] = ps_p.tile(
                                [128, 512], F32, tag=f"pp{qki}{ct}{mh}",
                                name=f"pq{qki}{ct}{mh}",
                            )
                for kb in range(8):
                    for qki, Wt in enumerate((WQ, WK)):
                        for ct in range(2):
                            for mh in range(2):
                                nc.tensor.matmul(
                                    pqs[(qki, ct, mh)][:],
                                    Wt[:, kb, 128 * ct : 128 * ct + 128],
                                    XT[:, kb, 512 * mh : 512 * mh + 512],
                                    start=(kb == 0),
                                    stop=(kb == 7),
                                )
                for qki in range(2):
                    for mh in range(2):
                        for ct in range(2):
                            evac[qki].tensor_copy(
                                QK8[:, ct, qki, 512 * mh : 512 * mh + 512],
                                pqs[(qki, ct, mh)][:],
                            )
                for mh in range(2):
                    for qki in range(2):
                        rt_dma(qki, mh)
                xs.add("pool", 3000)
                xs.add("dve", 3000)
            with tc.tile_pool(name="ps_v", bufs=1, space="PSUM") as ps_v:
                for mt in range(8):
                    v_group(ps_v, mt, pv_bufs=2)

            # ---- attention over the two query halves ----
            def attention(ic, pool):
                c0 = 512 * ic
                nkb = 4 * (ic + 1)
                DEN = pool.tile([128, 512], F32, tag="den", name=f"den{ic}")
                zero_mm(DEN[:])
                den_v = DEN[:, 0:64].rearrange("p (qb h) -> p qb h", h=HPG)
                CTXT = []
                for half in range(2):
                    t = pool.tile([128, 2, GCOLS], F32, tag="ctx", bufs=2,
                                  name=f"ctx{ic}{half}")
                    zero_mm(t[:])
                    CTXT.append(t)

                def ctx_of(qb):
                    return CTXT[qb // 2][:, qb % 2, :]

                def normalize(qb):
                    qq = 4 * ic + qb
                    ctx3 = ctx_of(qb).rearrange("p (h e) -> p h e", e=DH)
                    RG = work.tile([128, HPG], F32, tag="rg", name="rg")
                    nc.vector.reciprocal(out=RG[:], in_=den_v[:, qb, :])
                    nc.vector.tensor_tensor(
                        CN[:, qq, :].rearrange("p (h e) -> p h e", e=DH),
                        ctx3,
                        _bcast(RG[:], 2, DH),
                        op=MULT,
                    )
                    xs.add("dve", 900)

                def emit_ctx(kb, ATs):
                    for qb in range(max(0, kb - 4 * ic), 4):
                        qh, offc = qb // 2, 128 * (qb % 2)
                        for pr in range(8):
                            for ph in range(2):
                                h = 2 * pr + ph
                                lhsT = ATs[(pr, qh)][:, ph, offc : offc + 128]
                                nc.tensor.matmul(
                                    ctx_of(qb)[:, DH * h : DH * h + DH],
                                    lhsT,
                                    VA[:, kb, h, :],
                                    start=False,
                                    stop=False,
                                    skip_group_check=True,
                                )
                                nc.tensor.matmul(
                                    den_v[:, qb, h : h + 1],
                                    lhsT,
                                    ONES[:],
                                    start=False,
                                    stop=False,
                                    skip_group_check=True,
                                )

                prev = None
                for kb in range(nkb):
                    mhk, kbl = divmod(kb, 4)
                    lo = max(c0, 128 * kb)
                    j0 = lo - c0
                    diag = 128 * kb >= c0
                    qh_d = j0 // 256
                    ATs = {}
                    for pr in range(8):
                        for qh in range(qh_d, 2):
                            off = max(j0 - 256 * qh, 0)
                            SW = pool.tile([128, 2, 256], F32, tag="sw",
                                           bufs=SW_BUFS, name="sw")
                            for ph in range(2):
                                h = 2 * pr + ph
                                nc.tensor.matmul(
                                    SW[:, ph, off:256],
                                    QKT8h[mhk][:, 1, :, h,
                                               128 * kbl : 128 * kbl + 128],
                                    QKT8h[ic][:, 0, :, h,
                                              256 * qh + off : 256 * (qh + 1)],
                                    start=True,
                                    stop=not (diag and qh == qh_d),
                                    perf_mode=DR,
                                    skip_group_check=True,
                                )
                            if diag and qh == qh_d:
                                nc.tensor.matmul(
                                    SW[:, :, off : off + 128],
                                    ID2,
                                    MN2,
                                    start=False,
                                    stop=True,
                                    perf_mode=DR,
                                    skip_group_check=True,
                                )
                            AT = att.tile([128, 2, 256], BF16, tag="at",
                                          name="at")
                            xs.emit(
                                AT[:, :, off:256],
                                AT.bitcast(I16)[:, :, off:256],
                                SW[:, :, off:256],
                                2 * (256 - off),
                            )
                            ATs[(pr, qh)] = AT
                    if prev is not None:
                        emit_ctx(*prev)
                        if prev[0] - 4 * ic >= 0:
                            normalize(prev[0] - 4 * ic)
                    prev = (kb, ATs)
                emit_ctx(*prev)
                normalize(prev[0] - 4 * ic)

            with tc.tile_pool(name="ps_a0", bufs=1, space="PSUM") as ps_a0:
                attention(0, ps_a0)
            with tc.tile_pool(name="ps_a1", bufs=1, space="PSUM") as ps_a1:
                attention(1, ps_a1)

            # ---- P3: transpose + output projection ----
            ycp = [nc.scalar.copy, nc.vector.tensor_copy, nc.gpsimd.tensor_copy]
            with (
                tc.tile_pool(name="ps_t", bufs=2, space="PSUM") as ps_t,
                tc.tile_pool(name="ps_o", bufs=2, space="PSUM") as ps_o,
            ):
                for qq in range(8):
                    CT = work.tile([128, 2, 128], BF16, tag="ct", name="ct")
                    for ch in range(2):
                        TP = ps_t.tile([128, 128], BF16, tag="tp", name="tp")
                        nc.tensor.matmul(
                            TP[:],
                            CN[:, qq, 128 * ch : 128 * ch + 128],
                            IDENT[:],
                            is_transpose=True,
                        )
                        nc.vector.tensor_copy(CT[:, ch, :], TP[:])
                    PO = ps_o.tile([128, EMB], F32, tag="po", name="po")
                    for nh in range(2):
                        for ch in range(2):
                            nc.tensor.matmul(
                                PO[:, 512 * nh : 512 * nh + 512],
                                CT[:, ch, :],
                                WO[:, ch, 512 * nh : 512 * nh + 512],
                                start=(ch == 0),
                                stop=(ch == 1),
                            )
                    Y = work.tile([128, EMB], F32, tag="y", name="y")
                    for nh in range(2):
                        ycp[(2 * qq + nh) % 3](
                            Y[:, 512 * nh : 512 * nh + 512],
                            PO[:, 512 * nh : 512 * nh + 512],
                        )
                    nc.sync.dma_start(y_d[128 * qq : 128 * qq + 128, :], Y[:])

    split_excess_waits(nc)
    return nc


_NC_CACHE = None


def _get_nc():
    global _NC_CACHE
    if _NC_CACHE is None:
        _NC_CACHE = build_nc()
    return _NC_CACHE


# column permutation: device col j = 16*e + h  <-  module-local col 16*h + e
_PERM = [(j % 16) * 16 + j // 16 for j in range(GCOLS)]


def _consts():
    c8 = np.zeros((128, 2, 384), dtype=F8NP)
    j = np.arange(128)[:, None]
    m = np.arange(128)[None, :]
    mneg = np.where(j > m, np.float32(MNEG), np.float32(0.0))
    c8[:, 0, 0:128] = mneg.astype(F8NP)
    c8[:, 0, 128:256] = mneg.astype(F8NP)
    c8[:, 0, 256:384] = np.eye(128, dtype=np.float32).astype(F8NP)
    idm = np.eye(128, dtype=np.float32).astype(BF)
    return c8, idm


def kernel(x, Wq, Wk, Wv, Wo, bo):
    x = np.asarray(x, dtype=np.float32)
    Wq = np.asarray(Wq, dtype=np.float32)
    Wk = np.asarray(Wk, dtype=np.float32)
    Wv = np.asarray(Wv, dtype=np.float32)
    Wo = np.asarray(Wo, dtype=np.float32)
    bo = np.asarray(bo, dtype=np.float32)

    c8, idm = _consts()
    nc = _get_nc()
    in_maps = []
    for c in range(8):
        b, g = divmod(c, NG)
        cols = slice(GCOLS * g, GCOLS * g + GCOLS)
        in_maps.append(
            {
                "xT": np.ascontiguousarray(x[b].T).astype(BF),
                "wq": np.ascontiguousarray(Wq[:, cols][:, _PERM]).astype(BF),
                "wk": np.ascontiguousarray(Wk[:, cols][:, _PERM]).astype(BF),
                "wv": np.ascontiguousarray(Wv[:, cols]).astype(BF),
                "wo": np.ascontiguousarray(Wo[cols, :]).astype(BF),
                "c8": c8,
                "idm": idm,
            }
        )

    res = run_bass_kernel_spmd(nc, in_maps, core_ids=list(range(8)))
    out = np.zeros((BATCH, SEQ, EMB), dtype=np.float32)
    for c in range(8):
        b = c // NG
        out[b] += res.results[c]["y"]
    out += bo[None, None, :]
    return out


# revision 25
# speedup vs baseline: 1.4870x; 1.0069x over previous
"""Multi-head attention (axis-swapped variant) on 8 Trainium2 NeuronCores.

Reference semantics (EMB=1024): 64 effective heads of size 16 acting on the
d_head axis, causal softmax scaled by 1/sqrt(16), projections Wq/Wk/Wv,
output projection Wo + bo.

Sharding: core c = 4*b + g handles batch b and head-group g (16 heads =
256 contiguous projection columns). Each core returns a partial output
[1024, 1024]; the host sums the 4 group partials per batch and adds bo.

Per-core pipeline:
- bf16 Q/K/V projections (fp32 PSUM accumulate)
- Q/K evacuated to fp8e4m3 with columns pre-ordered (e,h); a DRAM
  roundtrip (parallel q/k chains on the ACT/DVE DMA queues) reshapes them
  to [8, qk, 2, 16, 512]-per-half so score matmuls run in fp8 DoubleRow
  mode (contraction 16 = 8 partitions x 2 k-tiles)
- causal diag masking as an extra fp8-DoubleRow matmul adding -120 above
  the diagonal (identity lhsT, precomputed mneg rhs) before the exp
- softmax exp split across three engines: ACT true exp, DVE/Pool use the
  Schraudolph bit-trick (y = int16(x*A+B) bitcast to bf16); score tiles
  are single-PSUM-bank [128, 2, 256] with 4 bufs for pipeline depth
- ctx accumulated transposed: out [128 queries, 16] per head, denominators
  via separate free-size-1 matmuls against a ones column
- normalize, PE transpose, bf16 out-projection
"""

import numpy as np
import ml_dtypes

import concourse.bass as bass
import concourse.mybir as mybir
import concourse.tile as tile
from concourse.bass_utils import run_bass_kernel_spmd

F32 = mybir.dt.float32
BF16 = mybir.dt.bfloat16
F8 = mybir.dt.float8e4
I16 = mybir.dt.int16
BF = ml_dtypes.bfloat16
F8NP = ml_dtypes.float8_e4m3

EMB = 1024
SEQ = 1024
BATCH = 2
NG = 4            # head groups (cores per batch)
HPG = 16          # heads per group/core
DH = 16           # per-head feature size
GCOLS = HPG * DH  # 256 projection columns per core

DR = mybir.MatmulPerfMode.DoubleRow
MULT = mybir.AluOpType.mult
ADD = mybir.AluOpType.add
EXPF = mybir.ActivationFunctionType.Exp

MNEG = -120.0
# Schraudolph exp: bf16(bitcast_int16(s * A_S + B_S)) ~= exp(0.25 * s)
A_S = float(np.float32(0.25 * 128.0 / np.log(2.0)))
B_S = 16248.0

N_WARM = 10  # PE p-state warmup matmuls while input DMAs land
SW_BUFS = 5
AT_BUFS = 40


def split_excess_waits(nc, cap=1):
    """This container's walrus rejects instructions carrying more than a few
    semaphore waits (and bass's own model says one). Relocate excess waits
    onto preceding same-engine EventSemaphore instructions."""

    def fix_block(bb, dummy):
        insts = bb.instructions
        i = 0
        while i < len(insts):
            inst = insts[i]
            si = inst.sync_info
            waits = list(si.on_wait) if si is not None and si.on_wait else []
            if len(waits) > cap:
                eng = nc.engines[inst.engine]
                excess, keep = waits[:-cap], waits[-cap:]
                si.on_wait = keep
                pos = i
                for j in range(0, len(excess), cap):
                    chunk = excess[j : j + cap]
                    ev = eng.wait_ge(dummy, 1)
                    cur_list = nc.cur_bb.bb.instructions
                    assert cur_list[-1] is ev.ins
                    cur_list.pop()
                    ev.ins.sync_info.on_wait = chunk
                    insts.insert(pos, ev.ins)
                    pos += 1
                    i += 1
            i += 1

    with nc.semaphore("waitfix_dummy") as dummy:
        for f in nc.m.functions:
            for bb in f.blocks:
                fix_block(bb, dummy)


def _bcast(ap, dim, count):
    """Insert a stride-0 dim at position `dim` of an AP."""
    new_ap = list(ap.ap)
    new_ap.insert(dim, [0, count])
    return bass.AP(tensor=ap.tensor, offset=ap.offset, ap=new_ap)


class ExpSplit:
    """Greedy load-balancing of exp work across ACT / DVE / Pool."""

    def __init__(self, nc):
        self.nc = nc
        # preload with approximate non-exp duties (ns)
        self.load = {"act": 2500.0, "dve": 12000.0, "pool": 9000.0}
        self.cost = {
            "act": lambda r: r * 0.8333 + 370.0,
            "dve": lambda r: r * 1.0417 + 260.0,
            "pool": lambda r: r * 1.389 + 140.0,
        }

    def emit(self, at, ati, sw, rows):
        eng = min(self.load, key=lambda e: self.load[e] + self.cost[e](rows))
        self.load[eng] += self.cost[eng](rows)
        if eng == "act":
            self.nc.scalar.activation(at, sw, EXPF, scale=0.25)
        elif eng == "dve":
            self.nc.vector.tensor_scalar(ati, sw, A_S, B_S, MULT, ADD)
        else:
            self.nc.gpsimd.tensor_scalar(ati, sw, A_S, B_S, MULT, ADD)

    def add(self, eng, ns):
        self.load[eng] += ns


def build_nc():
    nc = bass.Bass()
    xT_d = nc.declare_dram_parameter("xT", [EMB, SEQ], BF16, isOutput=False)
    wq_d = nc.declare_dram_parameter("wq", [EMB, GCOLS], BF16, isOutput=False)
    wk_d = nc.declare_dram_parameter("wk", [EMB, GCOLS], BF16, isOutput=False)
    wv_d = nc.declare_dram_parameter("wv", [EMB, GCOLS], BF16, isOutput=False)
    wo_d = nc.declare_dram_parameter("wo", [GCOLS, EMB], BF16, isOutput=False)
    c8_d = nc.declare_dram_parameter("c8", [128, 2, 384], F8, isOutput=False)
    id_d = nc.declare_dram_parameter("idm", [128, 128], BF16, isOutput=False)
    y_d = nc.declare_dram_parameter("y", [SEQ, EMB], F32, isOutput=True)

    with tile.TileContext(nc) as tc:
        with (
            tc.tile_pool(name="big", bufs=1) as big,
            tc.tile_pool(name="att", bufs=AT_BUFS) as att,
            tc.tile_pool(name="work", bufs=4) as work,
            tc.tile_pool(name="dram", bufs=1, space="DRAM") as dram,
        ):
            # ---- input DMAs (order = SP queue order; no waits on any) ----
            xT_r = xT_d[:].rearrange("(kb p) m -> p kb m", p=128)
            XT = big.tile([128, 8, SEQ], BF16)
            WQ = big.tile([128, 8, GCOLS], BF16)
            WK = big.tile([128, 8, GCOLS], BF16)
            WV = big.tile([128, 8, GCOLS], BF16)
            WO = big.tile([128, 2, EMB], BF16)
            C8 = big.tile([128, 2, 384], F8)
            IDENT = big.tile([128, 128], BF16)

            def xchunk(ci):
                sl = (slice(None), slice(2 * ci, 2 * ci + 2), slice(None))
                nc.sync.dma_start(XT[sl], xT_r[sl])

            nc.sync.dma_start(WQ[:], wq_d[:].rearrange("(kb p) n -> p kb n", p=128))
            xchunk(0)
            nc.sync.dma_start(WK[:], wk_d[:].rearrange("(kb p) n -> p kb n", p=128))
            xchunk(1)
            xchunk(2)
            xchunk(3)
            nc.sync.dma_start(WV[:], wv_d[:].rearrange("(kb p) n -> p kb n", p=128))
            nc.sync.dma_start(C8[:], c8_d[:])
            nc.sync.dma_start(WO[:], wo_d[:].rearrange("(ch p) n -> p ch n", p=128))
            nc.sync.dma_start(IDENT[:], id_d[:])

            MN2 = C8[:, :, 0:256]    # [p, i, (ph m)] additive -120 mask rhs
            ID2 = C8[:, :, 256:384]  # [p, i, j] identity pair lhsT

            QK8 = big.tile([128, 2, 2, SEQ], F8)       # (p=col, ct, qk, m)
            # per m-half fp8 score operands: (p8, qk, i, h, m)
            QKT8h = [
                big.tile([8, 2, 2, HPG, 512], F8, name=f"qkt8h{mh}")
                for mh in range(2)
            ]
            VA = big.tile([128, 8, HPG, DH], BF16)     # (p=key, kb, h, e)
            ONES = big.tile([128, 1], BF16)
            nc.gpsimd.memset(ONES[:], 1.0)
            ZL = big.tile([8, 2, 128], F8)
            nc.gpsimd.memset(ZL[:], 0.0)
            ZR = big.tile([8, 2, 512], F8)
            nc.gpsimd.memset(ZR[:], 0.0)
            CN = big.tile([128, 8, GCOLS], BF16)       # normalized ctx per qq
            qk8_d = dram.tile([2, 2, 2, 128, 512], F8)  # (mh, qk, ct, p, m)

            xs = ExpSplit(nc)
            evac = {0: nc.gpsimd, 1: nc.vector}   # qk evac: q->pool, k->dve

            def zero_mm(out_ap):
                nc.tensor.matmul(out_ap, ZL[:], ZR[:], start=True, stop=False,
                                 perf_mode=DR, skip_group_check=True)

            def rt_dma(qki, mh):
                nc.sync.dma_start(
                    qk8_d[mh, qki].rearrange("ct p m -> p ct m"),
                    QK8[:, :, qki, 512 * mh : 512 * mh + 512],
                )
                nc.sync.dma_start(
                    QKT8h[mh][:, qki],
                    qk8_d[mh, qki].rearrange("i (p8 h) m -> p8 i h m", p8=8),
                )

            def v_group(pool, mt, pv_bufs=1):
                pv = pool.tile([128, GCOLS], F32, tag="pv", name=f"pv{mt}",
                               bufs=pv_bufs)
                for kb in range(8):
                    nc.tensor.matmul(
                        pv[:],
                        XT[:, kb, 128 * mt : 128 * mt + 128],
                        WV[:, kb, :],
                        start=(kb == 0),
                        stop=(kb == 7),
                    )
                eng = nc.gpsimd if mt % 2 else nc.vector
                eng.tensor_copy(
                    VA[:, mt, :, :], pv[:].rearrange("p (h e) -> p h e", e=DH)
                )
                xs.add("pool" if mt % 2 else "dve", 420)

            # ---- P0: PE p-state warmup on zeros while DMAs land ----
            with tc.tile_pool(name="ps_w", bufs=1, space="PSUM") as ps_w:
                WARM = ps_w.tile([128, 512], F32, tag="warm")
                for _ in range(N_WARM):
                    nc.tensor.matmul(WARM[:], ZL[:], ZR[:], start=True, stop=True,
                                     perf_mode=DR, skip_group_check=True)

            # ---- P1a: Q/K proj both halves (8 pq banks), then V mt0..7 ----
            with tc.tile_pool(name="ps_p", bufs=1, space="PSUM") as ps_p:
                pqs = {}
                for qki in range(2):
                    for ct in range(2):
                        for mh in range(2):
                            pqs[(qki, ct, mh)] = ps_p.tile(
                                [128, 512], F32, tag=f"pp{qki}{ct}{mh}",
                                name=f"pq{qki}{ct}{mh}",
                            )
                for kb in range(8):
                    for qki, Wt in enumerate((WQ, WK)):
                        for ct in range(2):
                            for mh in range(2):
                                nc.tensor.matmul(
                                    pqs[(qki, ct, mh)][:],
                                    Wt[:, kb, 128 * ct : 128 * ct + 128],
                                    XT[:, kb, 512 * mh : 512 * mh + 512],
                                    start=(kb == 0),
                                    stop=(kb == 7),
                                )
                for qki in range(2):
                    for mh in range(2):
                        for ct in range(2):
                            evac[qki].tensor_copy(
                                QK8[:, ct, qki, 512 * mh : 512 * mh + 512],
                                pqs[(qki, ct, mh)][:],
                            )
                for mh in range(2):
                    for qki in range(2):
                        rt_dma(qki, mh)
                xs.add("pool", 3000)
                xs.add("dve", 3000)
            with tc.tile_pool(name="ps_v", bufs=1, space="PSUM") as ps_v:
                for mt in range(8):
                    v_group(ps_v, mt, pv_bufs=2)

            # ---- attention over the two query halves ----
            def attention(ic, pool):
                c0 = 512 * ic
                nkb = 4 * (ic + 1)
                DEN = pool.tile([128, 512], F32, tag="den", name=f"den{ic}")
                zero_mm(DEN[:])
                den_v = DEN[:, 0:64].rearrange("p (qb h) -> p qb h", h=HPG)
                CTXT = []
                for half in range(2):
                    t = pool.tile([128, 2, GCOLS], F32, tag="ctx", bufs=2,
                                  name=f"ctx{ic}{half}")
                    zero_mm(t[:])
                    CTXT.append(t)

                def ctx_of(qb):
                    return CTXT[qb // 2][:, qb % 2, :]

                def normalize(qb):
                    qq = 4 * ic + qb
                    ctx3 = ctx_of(qb).rearrange("p (h e) -> p h e", e=DH)
                    RG = work.tile([128, HPG], F32, tag="rg", name="rg")
                    nc.vector.reciprocal(out=RG[:], in_=den_v[:, qb, :])
                    nc.vector.tensor_tensor(
                        CN[:, qq, :].rearrange("p (h e) -> p h e", e=DH),
                        ctx3,
                        _bcast(RG[:], 2, DH),
                        op=MULT,
                    )
                    xs.add("dve", 900)

                def emit_ctx_pr(kb, ATs, pr):
                    for qb in range(max(0, kb - 4 * ic), 4):
                        qh, offc = qb // 2, 128 * (qb % 2)
                        for ph in range(2):
                            h = 2 * pr + ph
                            lhsT = ATs[(pr, qh)][:, ph, offc : offc + 128]
                            nc.tensor.matmul(
                                ctx_of(qb)[:, DH * h : DH * h + DH],
                                lhsT,
                                VA[:, kb, h, :],
                                start=False,
                                stop=False,
                                skip_group_check=True,
                            )
                            nc.tensor.matmul(
                                den_v[:, qb, h : h + 1],
                                lhsT,
                                ONES[:],
                                start=False,
                                stop=False,
                                skip_group_check=True,
                            )

                prev = None
                for kb in range(nkb):
                    mhk, kbl = divmod(kb, 4)
                    lo = max(c0, 128 * kb)
                    j0 = lo - c0
                    diag = 128 * kb >= c0
                    qh_d = j0 // 256
                    ATs = {}
                    for pr in range(8):
                        if prev is not None:
                            emit_ctx_pr(prev[0], prev[1], pr)
                        for qh in range(qh_d, 2):
                            off = max(j0 - 256 * qh, 0)
                            SW = pool.tile([128, 2, 256], F32, tag="sw",
                                           bufs=SW_BUFS, name="sw")
                            for ph in range(2):
                                h = 2 * pr + ph
                                nc.tensor.matmul(
                                    SW[:, ph, off:256],
                                    QKT8h[mhk][:, 1, :, h,
                                               128 * kbl : 128 * kbl + 128],
                                    QKT8h[ic][:, 0, :, h,
                                              256 * qh + off : 256 * (qh + 1)],
                                    start=True,
                                    stop=not (diag and qh == qh_d),
                                    perf_mode=DR,
                                    skip_group_check=True,
                                )
                            if diag and qh == qh_d:
                                nc.tensor.matmul(
                                    SW[:, :, off : off + 128],
                                    ID2,
                                    MN2,
                                    start=False,
                                    stop=True,
                                    perf_mode=DR,
                                    skip_group_check=True,
                                )
                            AT = att.tile([128, 2, 256], BF16, tag="at",
                                          name="at")
                            xs.emit(
                                AT[:, :, off:256],
                                AT.bitcast(I16)[:, :, off:256],
                                SW[:, :, off:256],
                                2 * (256 - off),
                            )
                            ATs[(pr, qh)] = AT
                    if prev is not None and prev[0] - 4 * ic >= 0:
                        normalize(prev[0] - 4 * ic)
                    prev = (kb, ATs)
                for pr in range(8):
                    emit_ctx_pr(prev[0], prev[1], pr)
                normalize(prev[0] - 4 * ic)

            with tc.tile_pool(name="ps_a0", bufs=1, space="PSUM") as ps_a0:
                attention(0, ps_a0)
            with tc.tile_pool(name="ps_a1", bufs=1, space="PSUM") as ps_a1:
                attention(1, ps_a1)

            # ---- P3: transpose + output projection ----
            ycp = [nc.scalar.copy, nc.vector.tensor_copy, nc.gpsimd.tensor_copy]
            with (
                tc.tile_pool(name="ps_t", bufs=2, space="PSUM") as ps_t,
                tc.tile_pool(name="ps_o", bufs=2, space="PSUM") as ps_o,
            ):
                for qq in range(8):
                    CT = work.tile([128, 2, 128], BF16, tag="ct", name="ct")
                    for ch in range(2):
                        TP = ps_t.tile([128, 128], BF16, tag="tp", name="tp")
                        nc.tensor.matmul(
                            TP[:],
                            CN[:, qq, 128 * ch : 128 * ch + 128],
                            IDENT[:],
                            is_transpose=True,
                        )
                        nc.vector.tensor_copy(CT[:, ch, :], TP[:])
                    PO = ps_o.tile([128, EMB], F32, tag="po", name="po")
                    for nh in range(2):
                        for ch in range(2):
                            nc.tensor.matmul(
                                PO[:, 512 * nh : 512 * nh + 512],
                                CT[:, ch, :],
                                WO[:, ch, 512 * nh : 512 * nh + 512],
                                start=(ch == 0),
                                stop=(ch == 1),
                            )
                    Y = work.tile([128, EMB], F32, tag="y", name="y")
                    for nh in range(2):
                        ycp[(2 * qq + nh) % 3](
                            Y[:, 512 * nh : 512 * nh + 512],
                            PO[:, 512 * nh : 512 * nh + 512],
                        )
                    nc.sync.dma_start(y_d[128 * qq : 128 * qq + 128, :], Y[:])

    split_excess_waits(nc)
    return nc


_NC_CACHE = None


def _get_nc():
    global _NC_CACHE
    if _NC_CACHE is None:
        _NC_CACHE = build_nc()
    return _NC_CACHE


# column permutation: device col j = 16*e + h  <-  module-local col 16*h + e
_PERM = [(j % 16) * 16 + j // 16 for j in range(GCOLS)]


def _consts():
    c8 = np.zeros((128, 2, 384), dtype=F8NP)
    j = np.arange(128)[:, None]
    m = np.arange(128)[None, :]
    mneg = np.where(j > m, np.float32(MNEG), np.float32(0.0))
    c8[:, 0, 0:128] = mneg.astype(F8NP)
    c8[:, 0, 128:256] = mneg.astype(F8NP)
    c8[:, 0, 256:384] = np.eye(128, dtype=np.float32).astype(F8NP)
    idm = np.eye(128, dtype=np.float32).astype(BF)
    return c8, idm


def kernel(x, Wq, Wk, Wv, Wo, bo):
    x = np.asarray(x, dtype=np.float32)
    Wq = np.asarray(Wq, dtype=np.float32)
    Wk = np.asarray(Wk, dtype=np.float32)
    Wv = np.asarray(Wv, dtype=np.float32)
    Wo = np.asarray(Wo, dtype=np.float32)
    bo = np.asarray(bo, dtype=np.float32)

    c8, idm = _consts()
    nc = _get_nc()
    in_maps = []
    for c in range(8):
        b, g = divmod(c, NG)
        cols = slice(GCOLS * g, GCOLS * g + GCOLS)
        in_maps.append(
            {
                "xT": np.ascontiguousarray(x[b].T).astype(BF),
                "wq": np.ascontiguousarray(Wq[:, cols][:, _PERM]).astype(BF),
                "wk": np.ascontiguousarray(Wk[:, cols][:, _PERM]).astype(BF),
                "wv": np.ascontiguousarray(Wv[:, cols]).astype(BF),
                "wo": np.ascontiguousarray(Wo[cols, :]).astype(BF),
                "c8": c8,
                "idm": idm,
            }
        )

    res = run_bass_kernel_spmd(nc, in_maps, core_ids=list(range(8)))
    out = np.zeros((BATCH, SEQ, EMB), dtype=np.float32)
    for c in range(8):
        b = c // NG
        out[b] += res.results[c]["y"]
    out += bo[None, None, :]
    return out
